# revision 21
# baseline (speedup 1.0000x reference)
"""Trainium2 Bass kernel for nn_Autoencoder_65223373357102 (FLAME-style autoencoder).

Strategy:
  Phase 1: encoder GEMM tensor-sharded along K across 8 cores; 145 dead latent
  columns dropped host-side -> [18816, 412] per core; fp32 matmuls (fp32r/bf16
  fail the 1/z sign-flip precision cliff). Ramped W/x chunking so PE starts
  ~10us in. A tiny warm-up AllReduce absorbs first-collective setup cost.
  AllReduce the [64,412] partial latents.
  Phase 2: V-sharded: each core computes+rotates+projects 440 of the 3520
  (padded) output verts. Two pipelined AllGathers (verts, then images) with
  dram->dram scatter into out overlapping the second. Eye processing collapsed
  to a [400,6] means-GEMM (rotated eye verts never reach the output); landmarks
  via a gathered [400, 3*74] mini-basis. Gaze/Cramer geometry hides under the
  AllGathers.
"""
import sys
import types

sys.path.insert(0, "/opt/trn_rl_repo")

import numpy as np


def _ensure_ntff_hook():
    """Provide antenv.axon_hooks + install the ctypes NTFF profile hook so
    run_bass_kernel_spmd(trace=True) can pull a neuron-profile under axon."""
    name = "antenv.axon_hooks"
    if name not in sys.modules:
        mod = types.ModuleType(name)
        mod._HOOK = None

        def set_axon_ntff_profile_hook(hook):
            mod._HOOK = hook

        def get_axon_ntff_profile_hook():
            return mod._HOOK

        mod.set_axon_ntff_profile_hook = set_axon_ntff_profile_hook
        mod.get_axon_ntff_profile_hook = get_axon_ntff_profile_hook
        sys.modules[name] = mod
        try:
            import antenv

            antenv.axon_hooks = mod
        except ImportError:
            pass
    mod = sys.modules[name]
    if mod.get_axon_ntff_profile_hook() is None:
        try:
            from trn_agent_boot.trn_boot import _ntff_profile_via_ctypes

            hook = _ntff_profile_via_ctypes("/opt/axon/libaxon_pjrt.so")
            if hook is not None:
                mod.set_axon_ntff_profile_hook(hook)
        except Exception:
            pass


_ensure_ntff_hook()

from concourse import bass, mybir, tile
from concourse.bass_utils import run_bass_kernel_spmd

F32 = mybir.dt.float32
ALU = mybir.AluOpType
ACTF = mybir.ActivationFunctionType
AX = mybir.AxisListType

B = 64
V = 5023
VM = 3500
DIN = 3 * 224 * 224  # 150528
NCORES = 8
KSH = DIN // NCORES  # 18816
KTILES = KSH // 128  # 147
WCH = [3, 6, 12] + [21] * 6  # ramped k-tiles per W chunk (sum 147)
XCH = [21, 42, 42, 42]       # x chunks (sum 147)
LIVE = 412           # 400 shape + 11 geo + 1 pad
NOUT = 2 * VM + 68 + 11  # 7079
VS = 440             # verts per core (8*440 = 3520 >= 3500)
NLM = 74             # 68 fl + 4 idx4 + 2 idx2
GAZE_DIR = -1.0
HALF_PI = 1.5707963267948966
# latent column remap (post live-column packing)
C_ROT, C_T, C_S, C_LROT, C_RROT = 400, 403, 406, 407, 409
# cst row layout
CB = 0            # 0:412 enc_b
CO8 = 412         # ones * 1/8
CO1 = 476         # ones * 1.0
CTM = 540         # tmpl mean (3)
CTE = 543         # tmpl eye means (6)
CTL = 549         # tmpl landmarks (222)
CSTW = 549 + 3 * NLM  # 771

KSPL = [(0, 128), (128, 128), (256, 128), (384, 16)]

_ENG_ATTR = {
    "SP": "sync", "Pool": "gpsimd", "PE": "tensor",
    "DVE": "vector", "Activation": "scalar",
}


def _legalize_waits(nc):
    """This walrus accepts only one sync-wait slot per instruction; move extra
    waits onto same-engine NoOps inserted right before the instruction."""
    import concourse.mybir as _mybir

    def make_nop(engine):
        eng = getattr(nc, _ENG_ATTR[engine.name])
        bi = eng.nop(nofuse=True)
        mi = bi.ins
        for bb in nc.main_func.blocks:
            if bb.instructions and bb.instructions[-1].name == mi.name:
                bb.instructions.pop()
                break
        mi.engine = engine
        return mi

    for bb in nc.main_func.blocks:
        snapshot = list(bb.instructions)
        newlist = []
        changed = False
        for inst in snapshot:
            si = inst.sync_info
            waits = list(si.on_wait) if (si and si.on_wait) else []
            if (
                len(waits) > 1
                and not inst.name.startswith("barrier")
                and inst.engine is not None
                and getattr(inst.engine, "name", None) in _ENG_ATTR
            ):
                for w in waits[:-1]:
                    nop = make_nop(inst.engine)
                    nop.sync_info = _mybir.SyncInfo(on_wait=[w], on_update=[])
                    newlist.append(nop)
                inst.sync_info = _mybir.SyncInfo(
                    on_wait=[waits[-1]], on_update=list(si.on_update)
                )
                changed = True
            newlist.append(inst)
        if changed:
            bb.instructions[:] = newlist


class Geo:
    """Helper for tiny per-batch scalar ops on [rows,1] tiles."""

    _uid = [0]

    def __init__(self, nc, pool, rows=B):
        self.nc = nc
        self.pool = pool
        self.rows = rows

    def t(self, cols=1):
        Geo._uid[0] += 1
        return self.pool.tile([self.rows, cols], F32, name=f"g{Geo._uid[0]}_{cols}")

    def mul(self, a, b):
        o = self.t()
        self.nc.vector.tensor_tensor(out=o, in0=a, in1=b, op=ALU.mult)
        return o

    def add(self, a, b):
        o = self.t()
        self.nc.vector.tensor_tensor(out=o, in0=a, in1=b, op=ALU.add)
        return o

    def sub(self, a, b):
        o = self.t()
        self.nc.vector.tensor_tensor(out=o, in0=a, in1=b, op=ALU.subtract)
        return o

    def mac(self, a, s, acc):
        """(a * s) + acc, s is a [rows,1] AP scalar."""
        o = self.t()
        self.nc.vector.scalar_tensor_tensor(
            out=o, in0=a, scalar=s, in1=acc, op0=ALU.mult, op1=ALU.add
        )
        return o

    def dot3(self, ax, ay, az, bx, by, bz):
        o = self.mul(ax, bx)
        o = self.mac(ay, by, o)
        o = self.mac(az, bz, o)
        return o

    def cross3(self, ax, ay, az, bx, by, bz):
        cx = self.sub(self.mul(ay, bz), self.mul(az, by))
        cy = self.sub(self.mul(az, bx), self.mul(ax, bz))
        cz = self.sub(self.mul(ax, by), self.mul(ay, bx))
        return cx, cy, cz


def axis_angle_R_multi(nc, jobs):
    """jobs: list of (g, aa3, pfx, halfpi_ap). ACT calls grouped by function to
    avoid table reloads. Returns list of R [rows,9] tiles, R[l,i] at col l*3+i."""
    st = []
    for (g, aa3, pfx, halfpi) in jobs:
        R_, pool = g.rows, g.pool
        sq = pool.tile([R_, 3], F32, name=pfx + "aaR_sq")
        nc.vector.tensor_tensor(out=sq, in0=aa3, in1=aa3, op=ALU.mult)
        th2 = g.t()
        nc.vector.tensor_reduce(out=th2, in_=sq, axis=AX.X, op=ALU.add)
        st.append({"g": g, "aa3": aa3, "pfx": pfx, "halfpi": halfpi, "th2": th2})
    for s_ in st:  # grouped Sqrt
        s_["theta"] = s_["g"].t()
        nc.scalar.activation(out=s_["theta"], in_=s_["th2"], func=ACTF.Sqrt)
    for s_ in st:  # grouped Sin (s and c back to back per job)
        g = s_["g"]
        s_["s"] = g.t()
        nc.scalar.activation(out=s_["s"], in_=s_["theta"], func=ACTF.Sin)
        s_["c"] = g.t()
        nc.scalar.activation(out=s_["c"], in_=s_["theta"], func=ACTF.Sin,
                             bias=s_["halfpi"])
    out = []
    for s_ in st:
        g = s_["g"]
        R_, pool, pfx = g.rows, g.pool, s_["pfx"]
        aa3, theta, s, c = s_["aa3"], s_["theta"], s_["s"], s_["c"]
        thm = g.t()
        nc.vector.tensor_scalar_max(out=thm, in0=theta, scalar1=1e-8)
        rth = g.t()
        nc.vector.reciprocal(out=rth, in_=thm)
        axis3 = pool.tile([R_, 3], F32, name=pfx + "aaR_axis")
        nc.vector.tensor_scalar_mul(out=axis3, in0=aa3, scalar1=rth)
        omc = g.t()
        nc.vector.tensor_scalar(
            out=omc, in0=c, scalar1=-1.0, scalar2=1.0, op0=ALU.mult, op1=ALU.add
        )
        ax, ay, az = axis3[:, 0:1], axis3[:, 1:2], axis3[:, 2:3]
        asq = pool.tile([R_, 3], F32, name=pfx + "aaR_asq")
        nc.vector.tensor_tensor(out=asq, in0=axis3, in1=axis3, op=ALU.mult)
        R = pool.tile([R_, 9], F32, name=pfx + "aaR_R")
        dmul = pool.tile([R_, 3], F32, name=pfx + "aaR_dmul")
        nc.vector.tensor_scalar_mul(out=dmul, in0=asq, scalar1=omc)
        sa = pool.tile([R_, 3], F32, name=pfx + "aaR_sa")
        nc.vector.tensor_scalar_mul(out=sa, in0=axis3, scalar1=s)
        sax, say, saz = sa[:, 0:1], sa[:, 1:2], sa[:, 2:3]
        mxy = g.mul(g.mul(ax, ay), omc)
        mxz = g.mul(g.mul(ax, az), omc)
        myz = g.mul(g.mul(ay, az), omc)
        for l in range(3):
            nc.vector.tensor_tensor(
                out=R[:, 4 * l:4 * l + 1], in0=dmul[:, l:l + 1], in1=c, op=ALU.add
            )
        nc.vector.tensor_tensor(out=R[:, 1:2], in0=mxy, in1=saz, op=ALU.subtract)
        nc.vector.tensor_tensor(out=R[:, 2:3], in0=mxz, in1=say, op=ALU.add)
        nc.vector.tensor_tensor(out=R[:, 3:4], in0=mxy, in1=saz, op=ALU.add)
        nc.vector.tensor_tensor(out=R[:, 5:6], in0=myz, in1=sax, op=ALU.subtract)
        nc.vector.tensor_tensor(out=R[:, 6:7], in0=mxz, in1=say, op=ALU.subtract)
        nc.vector.tensor_tensor(out=R[:, 7:8], in0=myz, in1=sax, op=ALU.add)
        out.append(R)
    return out


def _rotate3(nc, out3, in3, Rs, off, n):
    """out3[:, i, :n] = sum_l in3[:, l, :n]*Rs[l,i] + off_i  (all DVE)."""
    for i in range(3):
        nc.vector.tensor_scalar(
            out=out3[:, i, 0:n], in0=in3[:, 0, 0:n],
            scalar1=Rs[:, i:i + 1], scalar2=off[:, i:i + 1],
            op0=ALU.mult, op1=ALU.add,
        )
        for l in (1, 2):
            nc.vector.scalar_tensor_tensor(
                out=out3[:, i, 0:n], in0=in3[:, l, 0:n],
                scalar=Rs[:, 3 * l + i:3 * l + i + 1],
                in1=out3[:, i, 0:n], op0=ALU.mult, op1=ALU.add,
            )


def build_graph():
    nc = bass.Bass(target_bir_lowering=False)

    xp = nc.declare_dram_parameter("xp", [128, KTILES * B], F32, isOutput=False)
    wp = nc.declare_dram_parameter("wp", [128, KTILES * LIVE], F32, isOutput=False)
    cst_p = nc.declare_dram_parameter("cst", [1, CSTW], F32, isOutput=False)
    eye_p = nc.declare_dram_parameter("eye64", [B, B], F32, isOutput=False)
    bm_p = nc.declare_dram_parameter("bm", [128, 12], F32, isOutput=False)
    em_p = nc.declare_dram_parameter("em", [128, 24], F32, isOutput=False)
    lmb_p = nc.declare_dram_parameter("lmb", [128, 4 * 3 * NLM], F32, isOutput=False)
    bsl_p = nc.declare_dram_parameter("bsl", [128, 4 * 3 * VS], F32, isOutput=False)
    tsl_p = nc.declare_dram_parameter("tsl", [1, 3 * VS], F32, isOutput=False)
    cam_p = nc.declare_dram_parameter("cam", [B, 12], F32, isOutput=False)
    out_p = nc.declare_dram_parameter("out", [B, 3, NOUT], F32, isOutput=True)

    ar_in = nc.dram_tensor("ar_in", [B, LIVE], F32)
    ar_out = nc.dram_tensor("ar_out", [NCORES, B, LIVE], F32, addr_space="Shared")
    ag_in = nc.dram_tensor("ag_in", [B, 6, VS], F32)
    ag_out = nc.dram_tensor("ag_out", [NCORES, B, 6, VS], F32, addr_space="Shared")

    grp = [list(range(NCORES))]

    with tile.TileContext(nc) as tc:
        with (
            tc.tile_pool(name="consts", bufs=1) as consts,
            tc.tile_pool(name="latents", bufs=1) as latp,
            tc.tile_pool(name="geo", bufs=1) as geop,
            tc.tile_pool(name="minip", bufs=1, space="PSUM") as minip,
        ):
            # ---- small consts early (scalar queue) ----
            cst = consts.tile([1, CSTW], F32)
            nc.scalar.dma_start(out=cst, in_=cst_p[:, :])
            ones8 = cst[:, CO8:CO8 + B]
            ones1 = cst[:, CO1:CO1 + B]
            eye_sb = consts.tile([B, B], F32)
            nc.scalar.dma_start(out=eye_sb, in_=eye_p[:, :])
            bm_sb = consts.tile([128, 12], F32)
            nc.scalar.dma_start(out=bm_sb, in_=bm_p[:, :])
            em_sb = consts.tile([128, 24], F32)
            nc.scalar.dma_start(out=em_sb, in_=em_p[:, :])
            tsl_sb = consts.tile([1, 3 * VS], F32)
            nc.scalar.dma_start(out=tsl_sb, in_=tsl_p[:, :])
            cam = consts.tile([B, 12], F32)
            nc.scalar.dma_start(out=cam, in_=cam_p[:, :])
            halfpi = consts.tile([128, 1], F32)
            nc.vector.memset(halfpi, HALF_PI)

            # ---------------- Phase 1: encoder GEMM (fp32) ----------------
            NSPL = [(0, 412)]
            with (
                tc.tile_pool(name="xpool", bufs=1) as xpool,
                tc.tile_pool(name="wts", bufs=3) as wts,
                tc.tile_pool(name="encp", bufs=1, space="PSUM") as encp,
            ):
                x_sb = xpool.tile([128, KTILES * B], F32)
                pe = [encp.tile([B, n], F32, name=f"pe{j}", tag=f"pe{j}")
                      for j, (_, n) in enumerate(NSPL)]
                k = 0
                for wi, nk in enumerate(WCH):
                    w_c = wts.tile([128, 21 * LIVE], F32, name="wc", tag="wc")
                    weng = nc.sync if wi % 2 == 0 else nc.scalar
                    weng.dma_start(
                        out=x_sb[:, k * B:(k + nk) * B],
                        in_=xp[:, k * B:(k + nk) * B])
                    weng.dma_start(
                        out=w_c[:, 0:nk * LIVE],
                        in_=wp[:, k * LIVE:(k + nk) * LIVE])
                    for t in range(nk):
                        kk = k + t
                        for j, (n0, n) in enumerate(NSPL):
                            nc.tensor.matmul(
                                pe[j],
                                lhsT=x_sb[:, kk * B:(kk + 1) * B],
                                rhs=w_c[:, t * LIVE + n0:t * LIVE + n0 + n],
                                start=(kk == 0),
                                stop=False,
                            )
                    k += nk
                for j, (n0, n) in enumerate(NSPL):
                    nc.tensor.matmul(
                        pe[j], lhsT=ones8, rhs=cst[:, n0:n0 + n],
                        start=False, stop=True,
                    )
                lat1 = latp.tile([B, LIVE], F32)
                for j, (n0, n) in enumerate(NSPL):
                    nc.vector.tensor_copy(out=lat1[:, n0:n0 + n], in_=pe[j])
                nc.sync.dma_start(out=ar_in[:, :], in_=lat1)

            post_ctx = tc.tile_pool(name="post", bufs=1)
            post = post_ctx.__enter__()
            # deferred big loads on sync queue (start after last W chunk)
            lmb_sb = consts.tile([128, 4 * 3 * NLM], F32)
            nc.sync.dma_start(out=lmb_sb, in_=lmb_p[:, :])
            bsl_sb = consts.tile([128, 4, 3 * VS], F32)
            nc.sync.dma_start(
                out=bsl_sb, in_=bsl_p.ap().rearrange("p (c n) -> p c n", n=3 * VS))

            # PE-warm dummies: keep the HAM clock up during the collective
            dum = minip.tile([B, LIVE], F32, name="dum", tag="dum")
            for _ in range(40):
                nc.tensor.matmul(dum, lhsT=ones8, rhs=cst[:, 0:LIVE],
                                 start=True, stop=True, skip_group_check=True)
            nc.gpsimd.collective_compute(
                "AllGather", ALU.bypass, replica_groups=grp,
                ins=[ar_in.ap().opt()], outs=[ar_out.ap().opt()])
            slabs = post.tile([B, NCORES, LIVE], F32)
            nc.sync.dma_start(
                out=slabs, in_=ar_out.ap().rearrange("c b l -> b c l"))
            s4 = post.tile([B, 4, LIVE], F32)
            for c in range(4):
                nc.vector.tensor_tensor(
                    out=s4[:, c, :], in0=slabs[:, 2 * c, :],
                    in1=slabs[:, 2 * c + 1, :], op=ALU.add)
            lat = latp.tile([B, LIVE], F32)
            nc.vector.tensor_tensor(
                out=lat, in0=s4[:, 0, :], in1=s4[:, 1, :], op=ALU.add)
            nc.vector.tensor_tensor(
                out=lat, in0=lat, in1=s4[:, 2, :], op=ALU.add)
            nc.vector.tensor_tensor(
                out=lat, in0=lat, in1=s4[:, 3, :], op=ALU.add)

            # ---------------- spT via PE transpose ----------------
            spT = []
            with tc.tile_pool(name="trp", bufs=1, space="PSUM") as trp:
                for (c0, kw) in KSPL:
                    pt = trp.tile([128, B], F32, name=f"pt{c0}", tag=f"pt{c0}")
                    nc.tensor.transpose(
                        out=pt[0:kw, :], in_=lat[:, c0:c0 + kw], identity=eye_sb
                    )
                    st = latp.tile([128, B], F32, name=f"spT{c0}")
                    nc.vector.tensor_copy(out=st[0:kw, :], in_=pt[0:kw, :])
                    spT.append(st)

            g = Geo(nc, geop)
            g2 = Geo(nc, geop, rows=128)

            # ---------------- mini-GEMMs: vmean, eye means, landmarks ----------
            pvm = minip.tile([B, 3], F32, name="pvm", tag="pvm")
            pem = minip.tile([B, 6], F32, name="pem", tag="pem")
            plm = minip.tile([B, 3 * NLM], F32, name="plm", tag="plm")
            for ki, (k0, kw) in enumerate(KSPL):
                nc.tensor.matmul(pvm, lhsT=spT[ki][:kw, :], rhs=bm_sb[:kw, ki * 3:ki * 3 + 3],
                                 start=(ki == 0), stop=False)
            nc.tensor.matmul(pvm, lhsT=ones1, rhs=cst[:, CTM:CTM + 3],
                             start=False, stop=True)
            for ki, (k0, kw) in enumerate(KSPL):
                nc.tensor.matmul(pem, lhsT=spT[ki][:kw, :], rhs=em_sb[:kw, ki * 6:ki * 6 + 6],
                                 start=(ki == 0), stop=False)
            nc.tensor.matmul(pem, lhsT=ones1, rhs=cst[:, CTE:CTE + 6],
                             start=False, stop=True)
            for ki, (k0, kw) in enumerate(KSPL):
                nc.tensor.matmul(plm, lhsT=spT[ki][:kw, :],
                                 rhs=lmb_sb[:kw, ki * 3 * NLM:(ki + 1) * 3 * NLM],
                                 start=(ki == 0), stop=False)
            nc.tensor.matmul(plm, lhsT=ones1, rhs=cst[:, CTL:CTL + 3 * NLM],
                             start=False, stop=True)
            vms = geop.tile([B, 3], F32)
            nc.vector.tensor_copy(out=vms, in_=pvm)

            # ---------------- face rotation (critical path only) ---------------
            aa_face = lat[:, C_ROT:C_ROT + 3]
            [Rf] = axis_angle_R_multi(nc, [(g, aa_face, "f_", halfpi[:B, :])])
            fs = g.t()
            nc.vector.tensor_scalar_add(out=fs, in0=lat[:, C_S:C_S + 1], scalar1=1.0)
            Rs = geop.tile([B, 9], F32)
            nc.vector.tensor_scalar_mul(out=Rs, in0=Rf, scalar1=fs)
            off = geop.tile([B, 3], F32)
            for i in range(3):
                t = g.mul(vms[:, 0:1], Rs[:, i:i + 1])
                t = g.mac(vms[:, 1:2], Rs[:, 3 + i:4 + i], t)
                t = g.mac(vms[:, 2:3], Rs[:, 6 + i:7 + i], t)
                nc.vector.tensor_tensor(
                    out=off[:, i:i + 1], in0=lat[:, C_T + i:C_T + i + 1], in1=t,
                    op=ALU.subtract,
                )

            # ---------------- blendshape slice + rotate + project -------------
            pg = post.tile([B, 6, VS], F32)
            rt_c = pg[:, 0:3, :]
            vs_t = post.tile([B, 3, VS], F32)
            with tc.tile_pool(name="bpsum", bufs=3, space="PSUM") as bpsum:
                for p in range(3):
                    pv = bpsum.tile([B, VS], F32)
                    for ki, (k0, kw) in enumerate(KSPL):
                        nc.tensor.matmul(
                            pv, lhsT=spT[ki][:kw, :],
                            rhs=bsl_sb[:kw, ki, p * VS:(p + 1) * VS],
                            start=(ki == 0), stop=False,
                        )
                    nc.tensor.matmul(pv, lhsT=ones1, rhs=tsl_sb[:, p * VS:(p + 1) * VS],
                                     start=False, stop=True)
                    nc.vector.tensor_copy(out=vs_t[:, p, :], in_=pv)
                _rotate3(nc, rt_c, vs_t, Rs, off, VS)

            # projection of own slice into pg planes 3..5
            img_c = pg[:, 3:6, :]
            for i in (2, 0, 1):
                nc.vector.tensor_scalar(
                    out=img_c[:, i, :], in0=rt_c[:, 0, :],
                    scalar1=cam[:, 4 * i:4 * i + 1], scalar2=cam[:, 4 * i + 3:4 * i + 4],
                    op0=ALU.mult, op1=ALU.add,
                )
                for l in (1, 2):
                    nc.vector.scalar_tensor_tensor(
                        out=img_c[:, i, :], in0=rt_c[:, l, :],
                        scalar=cam[:, 4 * i + l:4 * i + l + 1],
                        in1=img_c[:, i, :], op0=ALU.mult, op1=ALU.add,
                    )
            az_ = post.tile([B, VS], F32)
            nc.scalar.activation(out=az_, in_=img_c[:, 2, :], func=ACTF.Abs)
            nc.vector.tensor_scalar_max(out=az_, in0=az_, scalar1=1e-3)
            sg = post.tile([B, VS], F32)
            nc.vector.tensor_scalar(
                out=sg, in0=img_c[:, 2, :], scalar1=0.0, scalar2=None, op0=ALU.is_ge
            )
            nc.vector.tensor_scalar(
                out=sg, in0=sg, scalar1=2.0, scalar2=1.0,
                op0=ALU.mult, op1=ALU.subtract,
            )
            nc.vector.tensor_tensor(out=sg, in0=sg, in1=az_, op=ALU.mult)
            nc.vector.reciprocal(out=az_, in_=sg)
            nc.vector.tensor_tensor(
                out=img_c[:, 0, :], in0=img_c[:, 0, :], in1=az_, op=ALU.mult
            )
            nc.vector.tensor_tensor(
                out=img_c[:, 1, :], in0=img_c[:, 1, :], in1=az_, op=ALU.mult
            )
            nc.sync.dma_start(out=ag_in[:, :, :], in_=pg)
            nc.gpsimd.collective_compute(
                "AllGather", ALU.bypass, replica_groups=grp,
                ins=[ag_in.ap().opt()], outs=[ag_out.ap().opt()])

            # ---------------- geometry (hides under AGs) ----------------------
            aa2 = geop.tile([128, 3], F32)
            nc.vector.memset(aa2, 0.0)
            nc.vector.tensor_copy(out=aa2[0:B, 0:2], in_=lat[:, C_LROT:C_LROT + 2])
            nc.gpsimd.dma_start(out=aa2[B:128, 0:2], in_=lat[:, C_RROT:C_RROT + 2])
            [R2] = axis_angle_R_multi(nc, [(g2, aa2, "e_", halfpi)])
            em_raw = geop.tile([B, 6], F32)
            nc.vector.tensor_copy(out=em_raw, in_=pem)
            lm_raw = geop.tile([B, 3, NLM], F32)
            nc.vector.tensor_copy(out=lm_raw, in_=plm)
            lm_t = geop.tile([B, 3, NLM], F32)
            _rotate3(nc, lm_t, lm_raw, Rs, off, NLM)
            for i in range(3):
                nc.scalar.dma_start(
                    out=out_p[:, i, 2 * VM:2 * VM + 68], in_=lm_t[:, i, 0:68]
                )
            fc = geop.tile([B, 3], F32)
            for i in range(3):
                t4 = g.add(lm_t[:, i, 68:69], lm_t[:, i, 69:70])
                t4 = g.add(t4, lm_t[:, i, 70:71])
                t4 = g.add(t4, lm_t[:, i, 71:72])
                t2 = g.add(lm_t[:, i, 72:73], lm_t[:, i, 73:74])
                o = g.t()
                nc.vector.tensor_scalar_mul(out=o, in0=t4, scalar1=0.125)
                nc.vector.scalar_tensor_tensor(
                    out=fc[:, i:i + 1], in0=t2, scalar=0.25, in1=o,
                    op0=ALU.mult, op1=ALU.add,
                )

            # eye centres: affine of raw means (stacked l/r on 128 rows)
            raw3 = geop.tile([128, 3], F32)
            nc.vector.tensor_copy(out=raw3[0:B, :], in_=em_raw[:, 0:3])
            nc.gpsimd.dma_start(out=raw3[B:128, :], in_=em_raw[:, 3:6])
            Rs128 = geop.tile([128, 9], F32)
            nc.vector.tensor_copy(out=Rs128[0:B, :], in_=Rs)
            nc.gpsimd.dma_start(out=Rs128[B:128, :], in_=Rs)
            off128 = geop.tile([128, 3], F32)
            nc.vector.tensor_copy(out=off128[0:B, :], in_=off)
            nc.gpsimd.dma_start(out=off128[B:128, :], in_=off)
            c3 = geop.tile([128, 3], F32)
            for i in range(3):
                o = g2.t()
                nc.vector.scalar_tensor_tensor(
                    out=o, in0=raw3[:, 0:1], scalar=Rs128[:, i:i + 1],
                    in1=off128[:, i:i + 1], op0=ALU.mult, op1=ALU.add,
                )
                o = g2.mac(raw3[:, 1:2], Rs128[:, 3 + i:4 + i], o)
                o = g2.mac(raw3[:, 2:3], Rs128[:, 6 + i:7 + i], o)
                nc.vector.tensor_copy(out=c3[:, i:i + 1], in_=o)

            gz = geop.tile([128, 3], F32)
            nc.vector.tensor_scalar_mul(out=gz, in0=R2[:, 6:9], scalar1=GAZE_DIR)
            rc64 = geop.tile([B, 3], F32)
            nc.gpsimd.dma_start(out=rc64, in_=c3[B:128, :])
            rg64 = geop.tile([B, 3], F32)
            nc.gpsimd.dma_start(out=rg64, in_=gz[B:128, :])
            lc = c3[0:B, :]
            lg = gz[0:B, :]
            rc = rc64
            rg = rg64

            # gaze intersection (Cramer)
            d = [g.sub(rc[:, i:i + 1], lc[:, i:i + 1]) for i in range(3)]
            c0 = [lg[:, i:i + 1] for i in range(3)]
            c1 = []
            for i in range(3):
                o = g.t()
                nc.vector.tensor_scalar_mul(out=o, in0=rg[:, i:i + 1], scalar1=-1.0)
                c1.append(o)
            c2 = list(g.cross3(rg[:, 0:1], rg[:, 1:2], rg[:, 2:3],
                               lg[:, 0:1], lg[:, 1:2], lg[:, 2:3]))
            w = g.cross3(*c1, *c2)
            det = g.dot3(*c0, *w)
            num0 = g.dot3(*d, *w)
            w2 = g.cross3(*d, *c2)
            num1 = g.dot3(*c0, *w2)
            rdet = g.t()
            nc.vector.reciprocal(out=rdet, in_=det)
            sol0 = g.mul(num0, rdet)
            sol1 = g.mul(num1, rdet)
            gpl = geop.tile([B, 3], F32)
            gpr = geop.tile([B, 3], F32)
            gpm = geop.tile([B, 3], F32)
            for i in range(3):
                nc.vector.scalar_tensor_tensor(
                    out=gpl[:, i:i + 1], in0=lg[:, i:i + 1], scalar=sol0,
                    in1=lc[:, i:i + 1], op0=ALU.mult, op1=ALU.add,
                )
                nc.vector.scalar_tensor_tensor(
                    out=gpr[:, i:i + 1], in0=rg[:, i:i + 1], scalar=sol1,
                    in1=rc[:, i:i + 1], op0=ALU.mult, op1=ALU.add,
                )
            nc.vector.tensor_tensor(out=gpm, in0=gpl, in1=gpr, op=ALU.add)
            nc.vector.tensor_scalar_mul(out=gpm, in0=gpm, scalar1=0.5)
            dff = geop.tile([B, 3], F32)
            nc.vector.tensor_tensor(out=dff, in0=gpl, in1=gpr, op=ALU.subtract)
            nc.vector.tensor_tensor(out=dff, in0=dff, in1=dff, op=ALU.mult)
            d2 = g.t()
            nc.vector.tensor_reduce(out=d2, in_=dff, axis=AX.X, op=ALU.add)
            dist = g.t()
            nc.scalar.activation(out=dist, in_=d2, func=ACTF.Sqrt)
            farl = geop.tile([B, 3], F32)
            farr = geop.tile([B, 3], F32)
            for i in range(3):
                nc.vector.scalar_tensor_tensor(
                    out=farl[:, i:i + 1], in0=lg[:, i:i + 1], scalar=1000.0,
                    in1=lc[:, i:i + 1], op0=ALU.mult, op1=ALU.add,
                )
                nc.vector.scalar_tensor_tensor(
                    out=farr[:, i:i + 1], in0=rg[:, i:i + 1], scalar=1000.0,
                    in1=rc[:, i:i + 1], op0=ALU.mult, op1=ALU.add,
                )

            # tail assembly [B, 3, 11]
            tail = geop.tile([B, 3, 11], F32)

            def _cp(k, out, in_):
                e = k % 3
                if e == 0:
                    nc.vector.tensor_copy(out=out, in_=in_)
                elif e == 1:
                    nc.scalar.copy(out=out, in_=in_)
                else:
                    nc.gpsimd.tensor_copy(out=out, in_=in_)

            for i in range(3):
                pieces = [
                    lc[:, i:i + 1], rc[:, i:i + 1], fc[:, i:i + 1],
                    gpl[:, i:i + 1], gpr[:, i:i + 1], gpm[:, i:i + 1],
                    farl[:, i:i + 1], farr[:, i:i + 1],
                    lg[:, i:i + 1], rg[:, i:i + 1], dist,
                ]
                for j, src in enumerate(pieces):
                    _cp(i * 11 + j, tail[:, i, j:j + 1], src)
            for i in range(3):
                nc.scalar.dma_start(
                    out=out_p[:, i, 2 * VM + 68:NOUT], in_=tail[:, i, :]
                )

            # ---------------- scatter gathered slices dram->dram --------------
            for j, c in enumerate(range(NCORES)):
                c0_ = c * VS
                nb = min(VM, c0_ + VS) - c0_
                e1 = nc.sync if c % 2 == 0 else nc.scalar
                e2 = nc.scalar if c % 2 == 0 else nc.sync
                e1.dma_start(
                    out=out_p[:, :, c0_:c0_ + nb],
                    in_=ag_out.ap()[c][:, 0:3, 0:nb],
                )
                e2.dma_start(
                    out=out_p[:, :, VM + c0_:VM + c0_ + nb],
                    in_=ag_out.ap()[c][:, 3:6, 0:nb],
                )
            post_ctx.__exit__(None, None, None)
    _legalize_waits(nc)
    return nc


def _prep(inputs):
    x = np.ascontiguousarray(np.asarray(inputs["x"], np.float32).reshape(B, DIN))
    W = np.asarray(inputs["enc_W"], np.float32)
    b = np.asarray(inputs["enc_b"], np.float32)
    tmpl = np.asarray(inputs["v_template"], np.float32)        # [V, 3]
    basis = np.asarray(inputs["shape_basis"], np.float32)      # [400, V, 3]
    cam = np.ascontiguousarray(
        np.asarray(inputs["camera_parameters"], np.float32).reshape(B, 12))
    lm = np.asarray(inputs["landmarks"])
    mlm = np.asarray(inputs["masked_landmarks"])
    fmask = np.asarray(inputs["face_mask"])
    lmask = np.asarray(inputs["left_eyeball_mask"])
    rmask = np.asarray(inputs["right_eyeball_mask"])

    live = list(range(400)) + list(range(545, 556))
    Wl = np.concatenate([W[:, live], np.zeros((DIN, 1), np.float32)], axis=1)  # [DIN, 412]
    bl = np.concatenate([b[live], np.zeros(1, np.float32)])

    fl_idx = [int(fmask[i]) for i in mlm]
    idx4 = [int(lm[j]) for j in (19, 22, 25, 28)]
    idx2 = [int(lm[j]) for j in (14, 18)]
    lm_all = fl_idx + idx4 + idx2  # 74

    cst = np.zeros((1, CSTW), np.float32)
    cst[0, CB:CB + LIVE] = bl
    cst[0, CO8:CO8 + B] = 1.0 / NCORES
    cst[0, CO1:CO1 + B] = 1.0
    cst[0, CTM:CTM + 3] = tmpl.mean(axis=0)
    cst[0, CTE:CTE + 3] = tmpl[lmask].mean(axis=0)
    cst[0, CTE + 3:CTE + 6] = tmpl[rmask].mean(axis=0)
    cst[0, CTL:CTL + 3 * NLM] = tmpl[lm_all].T.reshape(-1)  # plane-major [3, 74]

    eye64 = np.eye(B, dtype=np.float32)

    bmean_full = basis.mean(axis=1)            # [400, 3]
    el = basis[:, lmask].mean(axis=1)
    er = basis[:, rmask].mean(axis=1)
    bm = np.zeros((128, 12), np.float32)
    em = np.zeros((128, 24), np.float32)
    lmb = np.zeros((128, 4 * 3 * NLM), np.float32)
    bas_lm = basis[:, lm_all].transpose(0, 2, 1).reshape(400, 3 * NLM)
    for ki, (k0, kw) in enumerate(KSPL):
        bm[:kw, ki * 3:ki * 3 + 3] = bmean_full[k0:k0 + kw]
        em[:kw, ki * 6:ki * 6 + 3] = el[k0:k0 + kw]
        em[:kw, ki * 6 + 3:ki * 6 + 6] = er[k0:k0 + kw]
        lmb[:kw, ki * 3 * NLM:(ki + 1) * 3 * NLM] = bas_lm[k0:k0 + kw]

    in_maps = []
    basis_pm = basis.transpose(0, 2, 1)  # [400, 3, V]
    tmpl_pm = tmpl.T                     # [3, V]
    for c in range(NCORES):
        k0 = c * KSH
        xs = x[:, k0:k0 + KSH].reshape(B, KTILES, 128).transpose(2, 1, 0)
        xpk = np.ascontiguousarray(xs.reshape(128, KTILES * B))
        ws = Wl[k0:k0 + KSH].reshape(KTILES, 128, LIVE).transpose(1, 0, 2)
        wpk = np.ascontiguousarray(ws.reshape(128, KTILES * LIVE))
        v0 = c * VS
        v1 = min(V, v0 + VS)
        nb = v1 - v0
        bsl = np.zeros((128, 4, 3, VS), np.float32)
        tsl = np.zeros((1, 3, VS), np.float32)
        for ki, (kk0, kw) in enumerate(KSPL):
            bsl[:kw, ki, :, :nb] = basis_pm[kk0:kk0 + kw, :, v0:v1]
        tsl[0, :, :nb] = tmpl_pm[:, v0:v1]
        in_maps.append({
            "xp": xpk,
            "wp": wpk,
            "cst": cst,
            "eye64": eye64,
            "bm": bm,
            "em": em,
            "lmb": lmb,
            "bsl": np.ascontiguousarray(bsl.reshape(128, 4 * 3 * VS)),
            "tsl": np.ascontiguousarray(tsl.reshape(1, 3 * VS)),
            "cam": cam,
        })
    return in_maps


def _run(inputs, trace=False):
    in_maps = _prep(inputs)
    nc = build_graph()
    res = run_bass_kernel_spmd(
        nc, in_maps, core_ids=list(range(NCORES)), trace=trace
    )
    out = res.results[0]["out"]  # [B, 3, NOUT]
    return np.ascontiguousarray(out.transpose(0, 2, 1)), res


def kernel(**inputs):
    out, _ = _run(inputs, trace=False)
    return out


# revision 23
# speedup vs baseline: 1.0399x; 1.0399x over previous
"""Trainium2 Bass kernel for nn_Autoencoder_65223373357102 (FLAME-style autoencoder).

Strategy:
  Phase 1: encoder GEMM tensor-sharded along K across 8 cores; 145 dead latent
  columns dropped host-side -> [18816, 412] per core; fp32 matmuls (fp32r/bf16
  fail the 1/z sign-flip precision cliff). Ramped W/x chunking so PE starts
  ~10us in. A tiny warm-up AllReduce absorbs first-collective setup cost.
  AllReduce the [64,412] partial latents.
  Phase 2: V-sharded: each core computes+rotates+projects 440 of the 3520
  (padded) output verts. Two pipelined AllGathers (verts, then images) with
  dram->dram scatter into out overlapping the second. Eye processing collapsed
  to a [400,6] means-GEMM (rotated eye verts never reach the output); landmarks
  via a gathered [400, 3*74] mini-basis. Gaze/Cramer geometry hides under the
  AllGathers.
"""
import sys
import types

sys.path.insert(0, "/opt/trn_rl_repo")

import numpy as np


def _ensure_ntff_hook():
    """Provide antenv.axon_hooks + install the ctypes NTFF profile hook so
    run_bass_kernel_spmd(trace=True) can pull a neuron-profile under axon."""
    name = "antenv.axon_hooks"
    if name not in sys.modules:
        mod = types.ModuleType(name)
        mod._HOOK = None

        def set_axon_ntff_profile_hook(hook):
            mod._HOOK = hook

        def get_axon_ntff_profile_hook():
            return mod._HOOK

        mod.set_axon_ntff_profile_hook = set_axon_ntff_profile_hook
        mod.get_axon_ntff_profile_hook = get_axon_ntff_profile_hook
        sys.modules[name] = mod
        try:
            import antenv

            antenv.axon_hooks = mod
        except ImportError:
            pass
    mod = sys.modules[name]
    if mod.get_axon_ntff_profile_hook() is None:
        try:
            from trn_agent_boot.trn_boot import _ntff_profile_via_ctypes

            hook = _ntff_profile_via_ctypes("/opt/axon/libaxon_pjrt.so")
            if hook is not None:
                mod.set_axon_ntff_profile_hook(hook)
        except Exception:
            pass


_ensure_ntff_hook()

from concourse import bass, mybir, tile
from concourse.bass_utils import run_bass_kernel_spmd

F32 = mybir.dt.float32
ALU = mybir.AluOpType
ACTF = mybir.ActivationFunctionType
AX = mybir.AxisListType

B = 64
V = 5023
VM = 3500
DIN = 3 * 224 * 224  # 150528
NCORES = 8
KSH = DIN // NCORES  # 18816
KTILES = KSH // 128  # 147
WCH = [3, 6, 12] + [21] * 6  # ramped k-tiles per W chunk (sum 147)
XCH = [21, 42, 42, 42]       # x chunks (sum 147)
LIVE = 412           # 400 shape + 11 geo + 1 pad
NOUT = 2 * VM + 68 + 11  # 7079
VS = 440             # verts per core (8*440 = 3520 >= 3500)
NLM = 74             # 68 fl + 4 idx4 + 2 idx2
GAZE_DIR = -1.0
HALF_PI = 1.5707963267948966
# latent column remap (post live-column packing)
C_ROT, C_T, C_S, C_LROT, C_RROT = 400, 403, 406, 407, 409
# cst row layout
CB = 0            # 0:412 enc_b
CO8 = 412         # ones * 1/8
CO1 = 476         # ones * 1.0
CTM = 540         # tmpl mean (3)
CTE = 543         # tmpl eye means (6)
CTL = 549         # tmpl landmarks (222)
CSTW = 549 + 3 * NLM  # 771

KSPL = [(0, 128), (128, 128), (256, 128), (384, 16)]

_ENG_ATTR = {
    "SP": "sync", "Pool": "gpsimd", "PE": "tensor",
    "DVE": "vector", "Activation": "scalar",
}


def _legalize_waits(nc):
    """This walrus accepts only one sync-wait slot per instruction; move extra
    waits onto same-engine NoOps inserted right before the instruction."""
    import concourse.mybir as _mybir

    def make_nop(engine):
        eng = getattr(nc, _ENG_ATTR[engine.name])
        bi = eng.nop(nofuse=True)
        mi = bi.ins
        for bb in nc.main_func.blocks:
            if bb.instructions and bb.instructions[-1].name == mi.name:
                bb.instructions.pop()
                break
        mi.engine = engine
        return mi

    for bb in nc.main_func.blocks:
        snapshot = list(bb.instructions)
        newlist = []
        changed = False
        for inst in snapshot:
            si = inst.sync_info
            waits = list(si.on_wait) if (si and si.on_wait) else []
            if (
                len(waits) > 1
                and not inst.name.startswith("barrier")
                and inst.engine is not None
                and getattr(inst.engine, "name", None) in _ENG_ATTR
            ):
                for w in waits[:-1]:
                    nop = make_nop(inst.engine)
                    nop.sync_info = _mybir.SyncInfo(on_wait=[w], on_update=[])
                    newlist.append(nop)
                inst.sync_info = _mybir.SyncInfo(
                    on_wait=[waits[-1]], on_update=list(si.on_update)
                )
                changed = True
            newlist.append(inst)
        if changed:
            bb.instructions[:] = newlist


class Geo:
    """Helper for tiny per-batch scalar ops on [rows,1] tiles."""

    _uid = [0]

    def __init__(self, nc, pool, rows=B):
        self.nc = nc
        self.pool = pool
        self.rows = rows

    def t(self, cols=1):
        Geo._uid[0] += 1
        return self.pool.tile([self.rows, cols], F32, name=f"g{Geo._uid[0]}_{cols}")

    def mul(self, a, b):
        o = self.t()
        self.nc.vector.tensor_tensor(out=o, in0=a, in1=b, op=ALU.mult)
        return o

    def add(self, a, b):
        o = self.t()
        self.nc.vector.tensor_tensor(out=o, in0=a, in1=b, op=ALU.add)
        return o

    def sub(self, a, b):
        o = self.t()
        self.nc.vector.tensor_tensor(out=o, in0=a, in1=b, op=ALU.subtract)
        return o

    def mac(self, a, s, acc):
        """(a * s) + acc, s is a [rows,1] AP scalar."""
        o = self.t()
        self.nc.vector.scalar_tensor_tensor(
            out=o, in0=a, scalar=s, in1=acc, op0=ALU.mult, op1=ALU.add
        )
        return o

    def dot3(self, ax, ay, az, bx, by, bz):
        o = self.mul(ax, bx)
        o = self.mac(ay, by, o)
        o = self.mac(az, bz, o)
        return o

    def cross3(self, ax, ay, az, bx, by, bz):
        cx = self.sub(self.mul(ay, bz), self.mul(az, by))
        cy = self.sub(self.mul(az, bx), self.mul(ax, bz))
        cz = self.sub(self.mul(ax, by), self.mul(ay, bx))
        return cx, cy, cz


def axis_angle_R_multi(nc, jobs):
    """jobs: list of (g, aa3, pfx, halfpi_ap). ACT calls grouped by function to
    avoid table reloads. Returns list of R [rows,9] tiles, R[l,i] at col l*3+i."""
    st = []
    for (g, aa3, pfx, halfpi) in jobs:
        R_, pool = g.rows, g.pool
        sq = pool.tile([R_, 3], F32, name=pfx + "aaR_sq")
        nc.vector.tensor_tensor(out=sq, in0=aa3, in1=aa3, op=ALU.mult)
        th2 = g.t()
        nc.vector.tensor_reduce(out=th2, in_=sq, axis=AX.X, op=ALU.add)
        st.append({"g": g, "aa3": aa3, "pfx": pfx, "halfpi": halfpi, "th2": th2})
    for s_ in st:  # grouped Sqrt
        s_["theta"] = s_["g"].t()
        nc.scalar.activation(out=s_["theta"], in_=s_["th2"], func=ACTF.Sqrt)
    for s_ in st:  # grouped Sin (s and c back to back per job)
        g = s_["g"]
        s_["s"] = g.t()
        nc.scalar.activation(out=s_["s"], in_=s_["theta"], func=ACTF.Sin)
        s_["c"] = g.t()
        nc.scalar.activation(out=s_["c"], in_=s_["theta"], func=ACTF.Sin,
                             bias=s_["halfpi"])
    out = []
    for s_ in st:
        g = s_["g"]
        R_, pool, pfx = g.rows, g.pool, s_["pfx"]
        aa3, theta, s, c = s_["aa3"], s_["theta"], s_["s"], s_["c"]
        thm = g.t()
        nc.vector.tensor_scalar_max(out=thm, in0=theta, scalar1=1e-8)
        rth = g.t()
        nc.vector.reciprocal(out=rth, in_=thm)
        axis3 = pool.tile([R_, 3], F32, name=pfx + "aaR_axis")
        nc.vector.tensor_scalar_mul(out=axis3, in0=aa3, scalar1=rth)
        omc = g.t()
        nc.vector.tensor_scalar(
            out=omc, in0=c, scalar1=-1.0, scalar2=1.0, op0=ALU.mult, op1=ALU.add
        )
        ax, ay, az = axis3[:, 0:1], axis3[:, 1:2], axis3[:, 2:3]
        asq = pool.tile([R_, 3], F32, name=pfx + "aaR_asq")
        nc.vector.tensor_tensor(out=asq, in0=axis3, in1=axis3, op=ALU.mult)
        R = pool.tile([R_, 9], F32, name=pfx + "aaR_R")
        dmul = pool.tile([R_, 3], F32, name=pfx + "aaR_dmul")
        nc.vector.tensor_scalar_mul(out=dmul, in0=asq, scalar1=omc)
        sa = pool.tile([R_, 3], F32, name=pfx + "aaR_sa")
        nc.vector.tensor_scalar_mul(out=sa, in0=axis3, scalar1=s)
        sax, say, saz = sa[:, 0:1], sa[:, 1:2], sa[:, 2:3]
        mxy = g.mul(g.mul(ax, ay), omc)
        mxz = g.mul(g.mul(ax, az), omc)
        myz = g.mul(g.mul(ay, az), omc)
        for l in range(3):
            nc.vector.tensor_tensor(
                out=R[:, 4 * l:4 * l + 1], in0=dmul[:, l:l + 1], in1=c, op=ALU.add
            )
        nc.vector.tensor_tensor(out=R[:, 1:2], in0=mxy, in1=saz, op=ALU.subtract)
        nc.vector.tensor_tensor(out=R[:, 2:3], in0=mxz, in1=say, op=ALU.add)
        nc.vector.tensor_tensor(out=R[:, 3:4], in0=mxy, in1=saz, op=ALU.add)
        nc.vector.tensor_tensor(out=R[:, 5:6], in0=myz, in1=sax, op=ALU.subtract)
        nc.vector.tensor_tensor(out=R[:, 6:7], in0=mxz, in1=say, op=ALU.subtract)
        nc.vector.tensor_tensor(out=R[:, 7:8], in0=myz, in1=sax, op=ALU.add)
        out.append(R)
    return out


def _rotate3(nc, out3, in3, Rs, off, n):
    """out3[:, i, :n] = sum_l in3[:, l, :n]*Rs[l,i] + off_i  (all DVE)."""
    for i in range(3):
        nc.vector.tensor_scalar(
            out=out3[:, i, 0:n], in0=in3[:, 0, 0:n],
            scalar1=Rs[:, i:i + 1], scalar2=off[:, i:i + 1],
            op0=ALU.mult, op1=ALU.add,
        )
        for l in (1, 2):
            nc.vector.scalar_tensor_tensor(
                out=out3[:, i, 0:n], in0=in3[:, l, 0:n],
                scalar=Rs[:, 3 * l + i:3 * l + i + 1],
                in1=out3[:, i, 0:n], op0=ALU.mult, op1=ALU.add,
            )


def build_graph():
    nc = bass.Bass(target_bir_lowering=False)

    xp = nc.declare_dram_parameter("xp", [128, KTILES * B], F32, isOutput=False)
    wp = nc.declare_dram_parameter("wp", [128, KTILES * LIVE], F32, isOutput=False)
    cst_p = nc.declare_dram_parameter("cst", [1, CSTW], F32, isOutput=False)
    eye_p = nc.declare_dram_parameter("eye64", [B, B], F32, isOutput=False)
    bm_p = nc.declare_dram_parameter("bm", [128, 12], F32, isOutput=False)
    em_p = nc.declare_dram_parameter("em", [128, 24], F32, isOutput=False)
    lmb_p = nc.declare_dram_parameter("lmb", [128, 4 * 3 * NLM], F32, isOutput=False)
    bsl_p = nc.declare_dram_parameter("bsl", [128, 4 * 3 * VS], F32, isOutput=False)
    tsl_p = nc.declare_dram_parameter("tsl", [1, 3 * VS], F32, isOutput=False)
    cam_p = nc.declare_dram_parameter("cam", [B, 12], F32, isOutput=False)
    out_p = nc.declare_dram_parameter("out", [B, 3, NOUT], F32, isOutput=True)

    ar_in = nc.dram_tensor("ar_in", [B, LIVE], F32)
    ar_out = nc.dram_tensor("ar_out", [NCORES, B, LIVE], F32, addr_space="Shared")
    ag_in = nc.dram_tensor("ag_in", [B, 6, VS], F32)
    ag_out = nc.dram_tensor("ag_out", [NCORES, B, 6, VS], F32, addr_space="Shared")

    grp = [list(range(NCORES))]

    with tile.TileContext(nc) as tc:
        with (
            tc.tile_pool(name="consts", bufs=1) as consts,
            tc.tile_pool(name="latents", bufs=1) as latp,
            tc.tile_pool(name="geo", bufs=1) as geop,
            tc.tile_pool(name="minip", bufs=1, space="PSUM") as minip,
        ):
            # ---- small consts early (scalar queue) ----
            cst = consts.tile([1, CSTW], F32)
            nc.scalar.dma_start(out=cst, in_=cst_p[:, :])
            ones8 = cst[:, CO8:CO8 + B]
            ones1 = cst[:, CO1:CO1 + B]
            eye_sb = consts.tile([B, B], F32)
            nc.scalar.dma_start(out=eye_sb, in_=eye_p[:, :])
            bm_sb = consts.tile([128, 12], F32)
            nc.scalar.dma_start(out=bm_sb, in_=bm_p[:, :])
            em_sb = consts.tile([128, 24], F32)
            nc.scalar.dma_start(out=em_sb, in_=em_p[:, :])
            tsl_sb = consts.tile([1, 3 * VS], F32)
            nc.scalar.dma_start(out=tsl_sb, in_=tsl_p[:, :])
            cam = consts.tile([B, 12], F32)
            nc.scalar.dma_start(out=cam, in_=cam_p[:, :])
            halfpi = consts.tile([128, 1], F32)
            nc.vector.memset(halfpi, HALF_PI)

            # PE pre-warm: fill the HAM activity window before the k-loop so
            # the first real matmuls run at full clock
            dum0 = minip.tile([B, LIVE], F32, name="dum", tag="dum")
            for _ in range(5):
                nc.tensor.matmul(dum0, lhsT=ones8, rhs=cst[:, 0:LIVE],
                                 start=True, stop=True, skip_group_check=True)

            # ---------------- Phase 1: encoder GEMM (fp32) ----------------
            NSPL = [(0, 412)]
            with (
                tc.tile_pool(name="xpool", bufs=1) as xpool,
                tc.tile_pool(name="wts", bufs=3) as wts,
                tc.tile_pool(name="encp", bufs=1, space="PSUM") as encp,
            ):
                x_sb = xpool.tile([128, KTILES * B], F32)
                pe = [encp.tile([B, n], F32, name=f"pe{j}", tag=f"pe{j}")
                      for j, (_, n) in enumerate(NSPL)]
                k = 0
                for wi, nk in enumerate(WCH):
                    w_c = wts.tile([128, 21 * LIVE], F32, name="wc", tag="wc")
                    weng = nc.sync if wi % 2 == 0 else nc.scalar
                    weng.dma_start(
                        out=x_sb[:, k * B:(k + nk) * B],
                        in_=xp[:, k * B:(k + nk) * B])
                    weng.dma_start(
                        out=w_c[:, 0:nk * LIVE],
                        in_=wp[:, k * LIVE:(k + nk) * LIVE])
                    for t in range(nk):
                        kk = k + t
                        for j, (n0, n) in enumerate(NSPL):
                            nc.tensor.matmul(
                                pe[j],
                                lhsT=x_sb[:, kk * B:(kk + 1) * B],
                                rhs=w_c[:, t * LIVE + n0:t * LIVE + n0 + n],
                                start=(kk == 0),
                                stop=False,
                            )
                    k += nk
                for j, (n0, n) in enumerate(NSPL):
                    nc.tensor.matmul(
                        pe[j], lhsT=ones8, rhs=cst[:, n0:n0 + n],
                        start=False, stop=True,
                    )
                lat1 = latp.tile([B, LIVE], F32)
                for j, (n0, n) in enumerate(NSPL):
                    nc.vector.tensor_copy(out=lat1[:, n0:n0 + n], in_=pe[j])
                nc.sync.dma_start(out=ar_in[:, :], in_=lat1)

            post_ctx = tc.tile_pool(name="post", bufs=1)
            post = post_ctx.__enter__()
            # deferred big loads on sync queue (start after last W chunk)
            lmb_sb = consts.tile([128, 4 * 3 * NLM], F32)
            nc.sync.dma_start(out=lmb_sb, in_=lmb_p[:, :])
            bsl_sb = consts.tile([128, 4, 3 * VS], F32)
            nc.sync.dma_start(
                out=bsl_sb, in_=bsl_p.ap().rearrange("p (c n) -> p c n", n=3 * VS))

            # PE-warm dummies: keep the HAM clock up during the collective
            dum = minip.tile([B, LIVE], F32, name="dum", tag="dum")
            for _ in range(40):
                nc.tensor.matmul(dum, lhsT=ones8, rhs=cst[:, 0:LIVE],
                                 start=True, stop=True, skip_group_check=True)
            nc.gpsimd.collective_compute(
                "AllGather", ALU.bypass, replica_groups=grp,
                ins=[ar_in.ap().opt()], outs=[ar_out.ap().opt()])
            slabs = post.tile([B, NCORES, LIVE], F32)
            nc.sync.dma_start(
                out=slabs, in_=ar_out.ap().rearrange("c b l -> b c l"))
            s4 = post.tile([B, 4, LIVE], F32)
            for c in range(4):
                nc.vector.tensor_tensor(
                    out=s4[:, c, :], in0=slabs[:, 2 * c, :],
                    in1=slabs[:, 2 * c + 1, :], op=ALU.add)
            lat = latp.tile([B, LIVE], F32)
            nc.vector.tensor_tensor(
                out=lat, in0=s4[:, 0, :], in1=s4[:, 1, :], op=ALU.add)
            nc.vector.tensor_tensor(
                out=lat, in0=lat, in1=s4[:, 2, :], op=ALU.add)
            nc.vector.tensor_tensor(
                out=lat, in0=lat, in1=s4[:, 3, :], op=ALU.add)

            # ---------------- spT via PE transpose ----------------
            spT = []
            with tc.tile_pool(name="trp", bufs=1, space="PSUM") as trp:
                for (c0, kw) in KSPL:
                    pt = trp.tile([128, B], F32, name=f"pt{c0}", tag=f"pt{c0}")
                    nc.tensor.transpose(
                        out=pt[0:kw, :], in_=lat[:, c0:c0 + kw], identity=eye_sb
                    )
                    st = latp.tile([128, B], F32, name=f"spT{c0}")
                    nc.vector.tensor_copy(out=st[0:kw, :], in_=pt[0:kw, :])
                    spT.append(st)

            g = Geo(nc, geop)
            g2 = Geo(nc, geop, rows=128)

            # ---------------- mini-GEMMs: vmean, eye means, landmarks ----------
            pvm = minip.tile([B, 3], F32, name="pvm", tag="pvm")
            pem = minip.tile([B, 6], F32, name="pem", tag="pem")
            plm = minip.tile([B, 3 * NLM], F32, name="plm", tag="plm")
            for ki, (k0, kw) in enumerate(KSPL):
                nc.tensor.matmul(pvm, lhsT=spT[ki][:kw, :], rhs=bm_sb[:kw, ki * 3:ki * 3 + 3],
                                 start=(ki == 0), stop=False)
            nc.tensor.matmul(pvm, lhsT=ones1, rhs=cst[:, CTM:CTM + 3],
                             start=False, stop=True)
            for ki, (k0, kw) in enumerate(KSPL):
                nc.tensor.matmul(pem, lhsT=spT[ki][:kw, :], rhs=em_sb[:kw, ki * 6:ki * 6 + 6],
                                 start=(ki == 0), stop=False)
            nc.tensor.matmul(pem, lhsT=ones1, rhs=cst[:, CTE:CTE + 6],
                             start=False, stop=True)
            for ki, (k0, kw) in enumerate(KSPL):
                nc.tensor.matmul(plm, lhsT=spT[ki][:kw, :],
                                 rhs=lmb_sb[:kw, ki * 3 * NLM:(ki + 1) * 3 * NLM],
                                 start=(ki == 0), stop=False)
            nc.tensor.matmul(plm, lhsT=ones1, rhs=cst[:, CTL:CTL + 3 * NLM],
                             start=False, stop=True)
            vms = geop.tile([B, 3], F32)
            nc.vector.tensor_copy(out=vms, in_=pvm)

            # ---------------- face rotation (critical path only) ---------------
            aa_face = lat[:, C_ROT:C_ROT + 3]
            [Rf] = axis_angle_R_multi(nc, [(g, aa_face, "f_", halfpi[:B, :])])
            fs = g.t()
            nc.vector.tensor_scalar_add(out=fs, in0=lat[:, C_S:C_S + 1], scalar1=1.0)
            Rs = geop.tile([B, 9], F32)
            nc.vector.tensor_scalar_mul(out=Rs, in0=Rf, scalar1=fs)
            off = geop.tile([B, 3], F32)
            for i in range(3):
                t = g.mul(vms[:, 0:1], Rs[:, i:i + 1])
                t = g.mac(vms[:, 1:2], Rs[:, 3 + i:4 + i], t)
                t = g.mac(vms[:, 2:3], Rs[:, 6 + i:7 + i], t)
                nc.vector.tensor_tensor(
                    out=off[:, i:i + 1], in0=lat[:, C_T + i:C_T + i + 1], in1=t,
                    op=ALU.subtract,
                )

            # ---------------- blendshape slice + rotate + project -------------
            pg = post.tile([B, 6, VS], F32)
            rt_c = pg[:, 0:3, :]
            vs_t = post.tile([B, 3, VS], F32)
            with tc.tile_pool(name="bpsum", bufs=3, space="PSUM") as bpsum:
                for p in range(3):
                    pv = bpsum.tile([B, VS], F32)
                    for ki, (k0, kw) in enumerate(KSPL):
                        nc.tensor.matmul(
                            pv, lhsT=spT[ki][:kw, :],
                            rhs=bsl_sb[:kw, ki, p * VS:(p + 1) * VS],
                            start=(ki == 0), stop=False,
                        )
                    nc.tensor.matmul(pv, lhsT=ones1, rhs=tsl_sb[:, p * VS:(p + 1) * VS],
                                     start=False, stop=True)
                    nc.vector.tensor_copy(out=vs_t[:, p, :], in_=pv)
                _rotate3(nc, rt_c, vs_t, Rs, off, VS)

            # projection of own slice into pg planes 3..5
            img_c = pg[:, 3:6, :]
            for i in (2, 0, 1):
                nc.vector.tensor_scalar(
                    out=img_c[:, i, :], in0=rt_c[:, 0, :],
                    scalar1=cam[:, 4 * i:4 * i + 1], scalar2=cam[:, 4 * i + 3:4 * i + 4],
                    op0=ALU.mult, op1=ALU.add,
                )
                for l in (1, 2):
                    nc.vector.scalar_tensor_tensor(
                        out=img_c[:, i, :], in0=rt_c[:, l, :],
                        scalar=cam[:, 4 * i + l:4 * i + l + 1],
                        in1=img_c[:, i, :], op0=ALU.mult, op1=ALU.add,
                    )
            az_ = post.tile([B, VS], F32)
            nc.scalar.activation(out=az_, in_=img_c[:, 2, :], func=ACTF.Abs)
            nc.vector.tensor_scalar_max(out=az_, in0=az_, scalar1=1e-3)
            sg = post.tile([B, VS], F32)
            nc.vector.tensor_scalar(
                out=sg, in0=img_c[:, 2, :], scalar1=0.0, scalar2=None, op0=ALU.is_ge
            )
            nc.vector.tensor_scalar(
                out=sg, in0=sg, scalar1=2.0, scalar2=1.0,
                op0=ALU.mult, op1=ALU.subtract,
            )
            nc.vector.tensor_tensor(out=sg, in0=sg, in1=az_, op=ALU.mult)
            nc.vector.reciprocal(out=az_, in_=sg)
            nc.vector.tensor_tensor(
                out=img_c[:, 0, :], in0=img_c[:, 0, :], in1=az_, op=ALU.mult
            )
            nc.vector.tensor_tensor(
                out=img_c[:, 1, :], in0=img_c[:, 1, :], in1=az_, op=ALU.mult
            )
            nc.sync.dma_start(out=ag_in[:, :, :], in_=pg)
            nc.gpsimd.collective_compute(
                "AllGather", ALU.bypass, replica_groups=grp,
                ins=[ag_in.ap().opt()], outs=[ag_out.ap().opt()])

            # ---------------- geometry (hides under AGs) ----------------------
            aa2 = geop.tile([128, 3], F32)
            nc.vector.memset(aa2, 0.0)
            nc.vector.tensor_copy(out=aa2[0:B, 0:2], in_=lat[:, C_LROT:C_LROT + 2])
            nc.gpsimd.dma_start(out=aa2[B:128, 0:2], in_=lat[:, C_RROT:C_RROT + 2])
            [R2] = axis_angle_R_multi(nc, [(g2, aa2, "e_", halfpi)])
            em_raw = geop.tile([B, 6], F32)
            nc.vector.tensor_copy(out=em_raw, in_=pem)
            lm_raw = geop.tile([B, 3, NLM], F32)
            nc.vector.tensor_copy(out=lm_raw, in_=plm)
            lm_t = geop.tile([B, 3, NLM], F32)
            _rotate3(nc, lm_t, lm_raw, Rs, off, NLM)
            for i in range(3):
                nc.scalar.dma_start(
                    out=out_p[:, i, 2 * VM:2 * VM + 68], in_=lm_t[:, i, 0:68]
                )
            fc = geop.tile([B, 3], F32)
            for i in range(3):
                t4 = g.add(lm_t[:, i, 68:69], lm_t[:, i, 69:70])
                t4 = g.add(t4, lm_t[:, i, 70:71])
                t4 = g.add(t4, lm_t[:, i, 71:72])
                t2 = g.add(lm_t[:, i, 72:73], lm_t[:, i, 73:74])
                o = g.t()
                nc.vector.tensor_scalar_mul(out=o, in0=t4, scalar1=0.125)
                nc.vector.scalar_tensor_tensor(
                    out=fc[:, i:i + 1], in0=t2, scalar=0.25, in1=o,
                    op0=ALU.mult, op1=ALU.add,
                )

            # eye centres: affine of raw means (stacked l/r on 128 rows)
            raw3 = geop.tile([128, 3], F32)
            nc.vector.tensor_copy(out=raw3[0:B, :], in_=em_raw[:, 0:3])
            nc.gpsimd.dma_start(out=raw3[B:128, :], in_=em_raw[:, 3:6])
            Rs128 = geop.tile([128, 9], F32)
            nc.vector.tensor_copy(out=Rs128[0:B, :], in_=Rs)
            nc.gpsimd.dma_start(out=Rs128[B:128, :], in_=Rs)
            off128 = geop.tile([128, 3], F32)
            nc.vector.tensor_copy(out=off128[0:B, :], in_=off)
            nc.gpsimd.dma_start(out=off128[B:128, :], in_=off)
            c3 = geop.tile([128, 3], F32)
            for i in range(3):
                o = g2.t()
                nc.vector.scalar_tensor_tensor(
                    out=o, in0=raw3[:, 0:1], scalar=Rs128[:, i:i + 1],
                    in1=off128[:, i:i + 1], op0=ALU.mult, op1=ALU.add,
                )
                o = g2.mac(raw3[:, 1:2], Rs128[:, 3 + i:4 + i], o)
                o = g2.mac(raw3[:, 2:3], Rs128[:, 6 + i:7 + i], o)
                nc.vector.tensor_copy(out=c3[:, i:i + 1], in_=o)

            gz = geop.tile([128, 3], F32)
            nc.vector.tensor_scalar_mul(out=gz, in0=R2[:, 6:9], scalar1=GAZE_DIR)
            rc64 = geop.tile([B, 3], F32)
            nc.gpsimd.dma_start(out=rc64, in_=c3[B:128, :])
            rg64 = geop.tile([B, 3], F32)
            nc.gpsimd.dma_start(out=rg64, in_=gz[B:128, :])
            lc = c3[0:B, :]
            lg = gz[0:B, :]
            rc = rc64
            rg = rg64

            # gaze intersection (Cramer)
            d = [g.sub(rc[:, i:i + 1], lc[:, i:i + 1]) for i in range(3)]
            c0 = [lg[:, i:i + 1] for i in range(3)]
            c1 = []
            for i in range(3):
                o = g.t()
                nc.vector.tensor_scalar_mul(out=o, in0=rg[:, i:i + 1], scalar1=-1.0)
                c1.append(o)
            c2 = list(g.cross3(rg[:, 0:1], rg[:, 1:2], rg[:, 2:3],
                               lg[:, 0:1], lg[:, 1:2], lg[:, 2:3]))
            w = g.cross3(*c1, *c2)
            det = g.dot3(*c0, *w)
            num0 = g.dot3(*d, *w)
            w2 = g.cross3(*d, *c2)
            num1 = g.dot3(*c0, *w2)
            rdet = g.t()
            nc.vector.reciprocal(out=rdet, in_=det)
            sol0 = g.mul(num0, rdet)
            sol1 = g.mul(num1, rdet)
            gpl = geop.tile([B, 3], F32)
            gpr = geop.tile([B, 3], F32)
            gpm = geop.tile([B, 3], F32)
            for i in range(3):
                nc.vector.scalar_tensor_tensor(
                    out=gpl[:, i:i + 1], in0=lg[:, i:i + 1], scalar=sol0,
                    in1=lc[:, i:i + 1], op0=ALU.mult, op1=ALU.add,
                )
                nc.vector.scalar_tensor_tensor(
                    out=gpr[:, i:i + 1], in0=rg[:, i:i + 1], scalar=sol1,
                    in1=rc[:, i:i + 1], op0=ALU.mult, op1=ALU.add,
                )
            nc.vector.tensor_tensor(out=gpm, in0=gpl, in1=gpr, op=ALU.add)
            nc.vector.tensor_scalar_mul(out=gpm, in0=gpm, scalar1=0.5)
            dff = geop.tile([B, 3], F32)
            nc.vector.tensor_tensor(out=dff, in0=gpl, in1=gpr, op=ALU.subtract)
            nc.vector.tensor_tensor(out=dff, in0=dff, in1=dff, op=ALU.mult)
            d2 = g.t()
            nc.vector.tensor_reduce(out=d2, in_=dff, axis=AX.X, op=ALU.add)
            dist = g.t()
            nc.scalar.activation(out=dist, in_=d2, func=ACTF.Sqrt)
            farl = geop.tile([B, 3], F32)
            farr = geop.tile([B, 3], F32)
            for i in range(3):
                nc.vector.scalar_tensor_tensor(
                    out=farl[:, i:i + 1], in0=lg[:, i:i + 1], scalar=1000.0,
                    in1=lc[:, i:i + 1], op0=ALU.mult, op1=ALU.add,
                )
                nc.vector.scalar_tensor_tensor(
                    out=farr[:, i:i + 1], in0=rg[:, i:i + 1], scalar=1000.0,
                    in1=rc[:, i:i + 1], op0=ALU.mult, op1=ALU.add,
                )

            # tail assembly [B, 3, 11]
            tail = geop.tile([B, 3, 11], F32)

            def _cp(k, out, in_):
                e = k % 3
                if e == 0:
                    nc.vector.tensor_copy(out=out, in_=in_)
                elif e == 1:
                    nc.scalar.copy(out=out, in_=in_)
                else:
                    nc.gpsimd.tensor_copy(out=out, in_=in_)

            for i in range(3):
                pieces = [
                    lc[:, i:i + 1], rc[:, i:i + 1], fc[:, i:i + 1],
                    gpl[:, i:i + 1], gpr[:, i:i + 1], gpm[:, i:i + 1],
                    farl[:, i:i + 1], farr[:, i:i + 1],
                    lg[:, i:i + 1], rg[:, i:i + 1], dist,
                ]
                for j, src in enumerate(pieces):
                    _cp(i * 11 + j, tail[:, i, j:j + 1], src)
            for i in range(3):
                nc.scalar.dma_start(
                    out=out_p[:, i, 2 * VM + 68:NOUT], in_=tail[:, i, :]
                )

            # ---------------- scatter gathered slices dram->dram --------------
            for j, c in enumerate(range(NCORES)):
                c0_ = c * VS
                nb = min(VM, c0_ + VS) - c0_
                e1 = nc.sync if c % 2 == 0 else nc.scalar
                e2 = nc.scalar if c % 2 == 0 else nc.sync
                e1.dma_start(
                    out=out_p[:, :, c0_:c0_ + nb],
                    in_=ag_out.ap()[c][:, 0:3, 0:nb],
                )
                e2.dma_start(
                    out=out_p[:, :, VM + c0_:VM + c0_ + nb],
                    in_=ag_out.ap()[c][:, 3:6, 0:nb],
                )
            post_ctx.__exit__(None, None, None)
    _legalize_waits(nc)
    return nc


def _prep(inputs):
    x = np.ascontiguousarray(np.asarray(inputs["x"], np.float32).reshape(B, DIN))
    W = np.asarray(inputs["enc_W"], np.float32)
    b = np.asarray(inputs["enc_b"], np.float32)
    tmpl = np.asarray(inputs["v_template"], np.float32)        # [V, 3]
    basis = np.asarray(inputs["shape_basis"], np.float32)      # [400, V, 3]
    cam = np.ascontiguousarray(
        np.asarray(inputs["camera_parameters"], np.float32).reshape(B, 12))
    lm = np.asarray(inputs["landmarks"])
    mlm = np.asarray(inputs["masked_landmarks"])
    fmask = np.asarray(inputs["face_mask"])
    lmask = np.asarray(inputs["left_eyeball_mask"])
    rmask = np.asarray(inputs["right_eyeball_mask"])

    live = list(range(400)) + list(range(545, 556))
    Wl = np.concatenate([W[:, live], np.zeros((DIN, 1), np.float32)], axis=1)  # [DIN, 412]
    bl = np.concatenate([b[live], np.zeros(1, np.float32)])

    fl_idx = [int(fmask[i]) for i in mlm]
    idx4 = [int(lm[j]) for j in (19, 22, 25, 28)]
    idx2 = [int(lm[j]) for j in (14, 18)]
    lm_all = fl_idx + idx4 + idx2  # 74

    cst = np.zeros((1, CSTW), np.float32)
    cst[0, CB:CB + LIVE] = bl
    cst[0, CO8:CO8 + B] = 1.0 / NCORES
    cst[0, CO1:CO1 + B] = 1.0
    cst[0, CTM:CTM + 3] = tmpl.mean(axis=0)
    cst[0, CTE:CTE + 3] = tmpl[lmask].mean(axis=0)
    cst[0, CTE + 3:CTE + 6] = tmpl[rmask].mean(axis=0)
    cst[0, CTL:CTL + 3 * NLM] = tmpl[lm_all].T.reshape(-1)  # plane-major [3, 74]

    eye64 = np.eye(B, dtype=np.float32)

    bmean_full = basis.mean(axis=1)            # [400, 3]
    el = basis[:, lmask].mean(axis=1)
    er = basis[:, rmask].mean(axis=1)
    bm = np.zeros((128, 12), np.float32)
    em = np.zeros((128, 24), np.float32)
    lmb = np.zeros((128, 4 * 3 * NLM), np.float32)
    bas_lm = basis[:, lm_all].transpose(0, 2, 1).reshape(400, 3 * NLM)
    for ki, (k0, kw) in enumerate(KSPL):
        bm[:kw, ki * 3:ki * 3 + 3] = bmean_full[k0:k0 + kw]
        em[:kw, ki * 6:ki * 6 + 3] = el[k0:k0 + kw]
        em[:kw, ki * 6 + 3:ki * 6 + 6] = er[k0:k0 + kw]
        lmb[:kw, ki * 3 * NLM:(ki + 1) * 3 * NLM] = bas_lm[k0:k0 + kw]

    in_maps = []
    basis_pm = basis.transpose(0, 2, 1)  # [400, 3, V]
    tmpl_pm = tmpl.T                     # [3, V]
    for c in range(NCORES):
        k0 = c * KSH
        xs = x[:, k0:k0 + KSH].reshape(B, KTILES, 128).transpose(2, 1, 0)
        xpk = np.ascontiguousarray(xs.reshape(128, KTILES * B))
        ws = Wl[k0:k0 + KSH].reshape(KTILES, 128, LIVE).transpose(1, 0, 2)
        wpk = np.ascontiguousarray(ws.reshape(128, KTILES * LIVE))
        v0 = c * VS
        v1 = min(V, v0 + VS)
        nb = v1 - v0
        bsl = np.zeros((128, 4, 3, VS), np.float32)
        tsl = np.zeros((1, 3, VS), np.float32)
        for ki, (kk0, kw) in enumerate(KSPL):
            bsl[:kw, ki, :, :nb] = basis_pm[kk0:kk0 + kw, :, v0:v1]
        tsl[0, :, :nb] = tmpl_pm[:, v0:v1]
        in_maps.append({
            "xp": xpk,
            "wp": wpk,
            "cst": cst,
            "eye64": eye64,
            "bm": bm,
            "em": em,
            "lmb": lmb,
            "bsl": np.ascontiguousarray(bsl.reshape(128, 4 * 3 * VS)),
            "tsl": np.ascontiguousarray(tsl.reshape(1, 3 * VS)),
            "cam": cam,
        })
    return in_maps


def _run(inputs, trace=False):
    in_maps = _prep(inputs)
    nc = build_graph()
    res = run_bass_kernel_spmd(
        nc, in_maps, core_ids=list(range(NCORES)), trace=trace
    )
    out = res.results[0]["out"]  # [B, 3, NOUT]
    return np.ascontiguousarray(out.transpose(0, 2, 1)), res


def kernel(**inputs):
    out, _ = _run(inputs, trace=False)
    return out


# revision 26
# speedup vs baseline: 1.1372x; 1.0936x over previous
"""Trainium2 Bass kernel for nn_Autoencoder_65223373357102 (FLAME-style autoencoder).

Strategy:
  Phase 1: encoder GEMM tensor-sharded along K across 8 cores; 145 dead latent
  columns dropped host-side -> [18816, 412] per core; fp32 matmuls (fp32r/bf16
  fail the 1/z sign-flip precision cliff). Ramped W/x chunking so PE starts
  ~10us in. A tiny warm-up AllReduce absorbs first-collective setup cost.
  AllReduce the [64,412] partial latents.
  Phase 2: V-sharded: each core computes+rotates+projects 440 of the 3520
  (padded) output verts. Two pipelined AllGathers (verts, then images) with
  dram->dram scatter into out overlapping the second. Eye processing collapsed
  to a [400,6] means-GEMM (rotated eye verts never reach the output); landmarks
  via a gathered [400, 3*74] mini-basis. Gaze/Cramer geometry hides under the
  AllGathers.
"""
import sys
import types

sys.path.insert(0, "/opt/trn_rl_repo")

import numpy as np


def _ensure_ntff_hook():
    """Provide antenv.axon_hooks + install the ctypes NTFF profile hook so
    run_bass_kernel_spmd(trace=True) can pull a neuron-profile under axon."""
    name = "antenv.axon_hooks"
    if name not in sys.modules:
        mod = types.ModuleType(name)
        mod._HOOK = None

        def set_axon_ntff_profile_hook(hook):
            mod._HOOK = hook

        def get_axon_ntff_profile_hook():
            return mod._HOOK

        mod.set_axon_ntff_profile_hook = set_axon_ntff_profile_hook
        mod.get_axon_ntff_profile_hook = get_axon_ntff_profile_hook
        sys.modules[name] = mod
        try:
            import antenv

            antenv.axon_hooks = mod
        except ImportError:
            pass
    mod = sys.modules[name]
    if mod.get_axon_ntff_profile_hook() is None:
        try:
            from trn_agent_boot.trn_boot import _ntff_profile_via_ctypes

            hook = _ntff_profile_via_ctypes("/opt/axon/libaxon_pjrt.so")
            if hook is not None:
                mod.set_axon_ntff_profile_hook(hook)
        except Exception:
            pass


_ensure_ntff_hook()

from concourse import bass, mybir, tile
from concourse.bass_utils import run_bass_kernel_spmd

F32 = mybir.dt.float32
ALU = mybir.AluOpType
ACTF = mybir.ActivationFunctionType
AX = mybir.AxisListType

B = 64
V = 5023
VM = 3500
DIN = 3 * 224 * 224  # 150528
NCORES = 8
KSH = DIN // NCORES  # 18816
KTILES = KSH // 128  # 147
WCH = [3, 6, 12] + [21] * 6  # ramped k-tiles per W chunk (sum 147)
XCH = [21, 42, 42, 42]       # x chunks (sum 147)
LIVE = 412           # 400 shape + 11 geo + 1 pad
NOUT = 2 * VM + 68 + 11  # 7079
VS = 440             # verts per core (8*440 = 3520 >= 3500)
NLM = 74             # 68 fl + 4 idx4 + 2 idx2
GAZE_DIR = -1.0
HALF_PI = 1.5707963267948966
# latent column remap (post live-column packing)
C_ROT, C_T, C_S, C_LROT, C_RROT = 400, 403, 406, 407, 409
# cst row layout
CB = 0            # 0:412 enc_b
CO8 = 412         # ones * 1/8
CO1 = 476         # ones * 1.0
CTM = 540         # tmpl mean (3)
CTE = 543         # tmpl eye means (6)
CTL = 549         # tmpl landmarks (222)
CSTW = 549 + 3 * NLM  # 771

KSPL = [(0, 128), (128, 128), (256, 128), (384, 16)]

_ENG_ATTR = {
    "SP": "sync", "Pool": "gpsimd", "PE": "tensor",
    "DVE": "vector", "Activation": "scalar",
}


def _legalize_waits(nc):
    """This walrus accepts only one sync-wait slot per instruction; move extra
    waits onto same-engine NoOps inserted right before the instruction."""
    import concourse.mybir as _mybir

    def make_nop(engine):
        eng = getattr(nc, _ENG_ATTR[engine.name])
        bi = eng.nop(nofuse=True)
        mi = bi.ins
        for bb in nc.main_func.blocks:
            if bb.instructions and bb.instructions[-1].name == mi.name:
                bb.instructions.pop()
                break
        mi.engine = engine
        return mi

    for bb in nc.main_func.blocks:
        snapshot = list(bb.instructions)
        newlist = []
        changed = False
        for inst in snapshot:
            si = inst.sync_info
            waits = list(si.on_wait) if (si and si.on_wait) else []
            if (
                len(waits) > 1
                and not inst.name.startswith("barrier")
                and inst.engine is not None
                and getattr(inst.engine, "name", None) in _ENG_ATTR
            ):
                for w in waits[:-1]:
                    nop = make_nop(inst.engine)
                    nop.sync_info = _mybir.SyncInfo(on_wait=[w], on_update=[])
                    newlist.append(nop)
                inst.sync_info = _mybir.SyncInfo(
                    on_wait=[waits[-1]], on_update=list(si.on_update)
                )
                changed = True
            newlist.append(inst)
        if changed:
            bb.instructions[:] = newlist


class Geo:
    """Helper for tiny per-batch scalar ops on [rows,1] tiles."""

    _uid = [0]

    def __init__(self, nc, pool, rows=B):
        self.nc = nc
        self.pool = pool
        self.rows = rows

    def t(self, cols=1):
        Geo._uid[0] += 1
        return self.pool.tile([self.rows, cols], F32, name=f"g{Geo._uid[0]}_{cols}")

    def mul(self, a, b):
        o = self.t()
        self.nc.vector.tensor_tensor(out=o, in0=a, in1=b, op=ALU.mult)
        return o

    def add(self, a, b):
        o = self.t()
        self.nc.vector.tensor_tensor(out=o, in0=a, in1=b, op=ALU.add)
        return o

    def sub(self, a, b):
        o = self.t()
        self.nc.vector.tensor_tensor(out=o, in0=a, in1=b, op=ALU.subtract)
        return o

    def mac(self, a, s, acc):
        """(a * s) + acc, s is a [rows,1] AP scalar."""
        o = self.t()
        self.nc.vector.scalar_tensor_tensor(
            out=o, in0=a, scalar=s, in1=acc, op0=ALU.mult, op1=ALU.add
        )
        return o

    def dot3(self, ax, ay, az, bx, by, bz):
        o = self.mul(ax, bx)
        o = self.mac(ay, by, o)
        o = self.mac(az, bz, o)
        return o

    def cross3(self, ax, ay, az, bx, by, bz):
        cx = self.sub(self.mul(ay, bz), self.mul(az, by))
        cy = self.sub(self.mul(az, bx), self.mul(ax, bz))
        cz = self.sub(self.mul(ax, by), self.mul(ay, bx))
        return cx, cy, cz


def axis_angle_R_multi(nc, jobs):
    """jobs: list of (g, aa3, pfx, halfpi_ap). ACT calls grouped by function to
    avoid table reloads. Returns list of R [rows,9] tiles, R[l,i] at col l*3+i."""
    st = []
    for (g, aa3, pfx, halfpi) in jobs:
        R_, pool = g.rows, g.pool
        sq = pool.tile([R_, 3], F32, name=pfx + "aaR_sq")
        nc.vector.tensor_tensor(out=sq, in0=aa3, in1=aa3, op=ALU.mult)
        th2 = g.t()
        nc.vector.tensor_reduce(out=th2, in_=sq, axis=AX.X, op=ALU.add)
        st.append({"g": g, "aa3": aa3, "pfx": pfx, "halfpi": halfpi, "th2": th2})
    for s_ in st:  # grouped Sqrt
        s_["theta"] = s_["g"].t()
        nc.scalar.activation(out=s_["theta"], in_=s_["th2"], func=ACTF.Sqrt)
    for s_ in st:  # grouped Sin (s and c back to back per job)
        g = s_["g"]
        s_["s"] = g.t()
        nc.scalar.activation(out=s_["s"], in_=s_["theta"], func=ACTF.Sin)
        s_["c"] = g.t()
        nc.scalar.activation(out=s_["c"], in_=s_["theta"], func=ACTF.Sin,
                             bias=s_["halfpi"])
    out = []
    for s_ in st:
        g = s_["g"]
        R_, pool, pfx = g.rows, g.pool, s_["pfx"]
        aa3, theta, s, c = s_["aa3"], s_["theta"], s_["s"], s_["c"]
        thm = g.t()
        nc.vector.tensor_scalar_max(out=thm, in0=theta, scalar1=1e-8)
        rth = g.t()
        nc.vector.reciprocal(out=rth, in_=thm)
        axis3 = pool.tile([R_, 3], F32, name=pfx + "aaR_axis")
        nc.vector.tensor_scalar_mul(out=axis3, in0=aa3, scalar1=rth)
        omc = g.t()
        nc.vector.tensor_scalar(
            out=omc, in0=c, scalar1=-1.0, scalar2=1.0, op0=ALU.mult, op1=ALU.add
        )
        ax, ay, az = axis3[:, 0:1], axis3[:, 1:2], axis3[:, 2:3]
        asq = pool.tile([R_, 3], F32, name=pfx + "aaR_asq")
        nc.vector.tensor_tensor(out=asq, in0=axis3, in1=axis3, op=ALU.mult)
        R = pool.tile([R_, 9], F32, name=pfx + "aaR_R")
        dmul = pool.tile([R_, 3], F32, name=pfx + "aaR_dmul")
        nc.vector.tensor_scalar_mul(out=dmul, in0=asq, scalar1=omc)
        sa = pool.tile([R_, 3], F32, name=pfx + "aaR_sa")
        nc.vector.tensor_scalar_mul(out=sa, in0=axis3, scalar1=s)
        sax, say, saz = sa[:, 0:1], sa[:, 1:2], sa[:, 2:3]
        mxy = g.mul(g.mul(ax, ay), omc)
        mxz = g.mul(g.mul(ax, az), omc)
        myz = g.mul(g.mul(ay, az), omc)
        for l in range(3):
            nc.vector.tensor_tensor(
                out=R[:, 4 * l:4 * l + 1], in0=dmul[:, l:l + 1], in1=c, op=ALU.add
            )
        nc.vector.tensor_tensor(out=R[:, 1:2], in0=mxy, in1=saz, op=ALU.subtract)
        nc.vector.tensor_tensor(out=R[:, 2:3], in0=mxz, in1=say, op=ALU.add)
        nc.vector.tensor_tensor(out=R[:, 3:4], in0=mxy, in1=saz, op=ALU.add)
        nc.vector.tensor_tensor(out=R[:, 5:6], in0=myz, in1=sax, op=ALU.subtract)
        nc.vector.tensor_tensor(out=R[:, 6:7], in0=mxz, in1=say, op=ALU.subtract)
        nc.vector.tensor_tensor(out=R[:, 7:8], in0=myz, in1=sax, op=ALU.add)
        out.append(R)
    return out


def _rotate3(nc, out3, in3, Rs, off, n):
    """out3[:, i, :n] = sum_l in3[:, l, :n]*Rs[l,i] + off_i  (all DVE)."""
    for i in range(3):
        nc.vector.tensor_scalar(
            out=out3[:, i, 0:n], in0=in3[:, 0, 0:n],
            scalar1=Rs[:, i:i + 1], scalar2=off[:, i:i + 1],
            op0=ALU.mult, op1=ALU.add,
        )
        for l in (1, 2):
            nc.vector.scalar_tensor_tensor(
                out=out3[:, i, 0:n], in0=in3[:, l, 0:n],
                scalar=Rs[:, 3 * l + i:3 * l + i + 1],
                in1=out3[:, i, 0:n], op0=ALU.mult, op1=ALU.add,
            )


def build_graph():
    nc = bass.Bass(target_bir_lowering=False)

    xp = nc.declare_dram_parameter("xp", [128, KTILES * B], F32, isOutput=False)
    wp = nc.declare_dram_parameter("wp", [128, KTILES * LIVE], F32, isOutput=False)
    cst_p = nc.declare_dram_parameter("cst", [1, CSTW], F32, isOutput=False)
    eye_p = nc.declare_dram_parameter("eye64", [B, B], F32, isOutput=False)
    bm_p = nc.declare_dram_parameter("bm", [128, 12], F32, isOutput=False)
    em_p = nc.declare_dram_parameter("em", [128, 24], F32, isOutput=False)
    lmb_p = nc.declare_dram_parameter("lmb", [128, 4 * 3 * NLM], F32, isOutput=False)
    bsl_p = nc.declare_dram_parameter("bsl", [128, 4 * 3 * VS], F32, isOutput=False)
    tsl_p = nc.declare_dram_parameter("tsl", [1, 3 * VS], F32, isOutput=False)
    cam_p = nc.declare_dram_parameter("cam", [B, 12], F32, isOutput=False)
    out_p = nc.declare_dram_parameter("out", [B, 3, NOUT], F32, isOutput=True)

    ar_in = nc.dram_tensor("ar_in", [B, LIVE], F32)
    ar_out = nc.dram_tensor("ar_out", [NCORES, B, LIVE], F32, addr_space="Shared")
    ag_in = nc.dram_tensor("ag_in", [B, 6, VS], F32)
    ag_out = nc.dram_tensor("ag_out", [NCORES, B, 6, VS], F32, addr_space="Shared")

    grp = [list(range(NCORES))]

    with tile.TileContext(nc) as tc:
        with (
            tc.tile_pool(name="consts", bufs=1) as consts,
            tc.tile_pool(name="latents", bufs=1) as latp,
            tc.tile_pool(name="geo", bufs=1) as geop,
            tc.tile_pool(name="minip", bufs=1, space="PSUM") as minip,
        ):
            # ---- small consts early (scalar queue) ----
            cst = consts.tile([1, CSTW], F32)
            nc.scalar.dma_start(out=cst, in_=cst_p[:, :])
            ones8 = cst[:, CO8:CO8 + B]
            ones1 = cst[:, CO1:CO1 + B]
            eye_sb = consts.tile([B, B], F32)
            nc.scalar.dma_start(out=eye_sb, in_=eye_p[:, :])
            bm_sb = consts.tile([128, 12], F32)
            nc.scalar.dma_start(out=bm_sb, in_=bm_p[:, :])
            em_sb = consts.tile([128, 24], F32)
            nc.scalar.dma_start(out=em_sb, in_=em_p[:, :])
            cam = consts.tile([B, 12], F32)
            nc.scalar.dma_start(out=cam, in_=cam_p[:, :])
            halfpi = consts.tile([128, 1], F32)
            nc.vector.memset(halfpi, HALF_PI)

            # ---------------- Phase 1: encoder GEMM (fp32) ----------------
            NSPL = [(0, 412)]
            with (
                tc.tile_pool(name="xpool", bufs=1) as xpool,
                tc.tile_pool(name="wts", bufs=4) as wts,
                tc.tile_pool(name="encp", bufs=1, space="PSUM") as encp,
            ):
                x_sb = xpool.tile([128, KTILES * B], F32)
                pe = [encp.tile([B, n], F32, name=f"pe{j}", tag=f"pe{j}")
                      for j, (_, n) in enumerate(NSPL)]
                k = 0
                for wi, nk in enumerate(WCH):
                    w_c = wts.tile([128, 21 * LIVE], F32, name="wc", tag="wc")
                    weng = nc.sync if wi % 2 == 0 else nc.scalar
                    weng.dma_start(
                        out=x_sb[:, k * B:(k + nk) * B],
                        in_=xp[:, k * B:(k + nk) * B])
                    weng.dma_start(
                        out=w_c[:, 0:nk * LIVE],
                        in_=wp[:, k * LIVE:(k + nk) * LIVE])
                    for t in range(nk):
                        kk = k + t
                        for j, (n0, n) in enumerate(NSPL):
                            nc.tensor.matmul(
                                pe[j],
                                lhsT=x_sb[:, kk * B:(kk + 1) * B],
                                rhs=w_c[:, t * LIVE + n0:t * LIVE + n0 + n],
                                start=(kk == 0),
                                stop=False,
                            )
                    k += nk
                for j, (n0, n) in enumerate(NSPL):
                    nc.tensor.matmul(
                        pe[j], lhsT=ones8, rhs=cst[:, n0:n0 + n],
                        start=False, stop=True,
                    )
                lat1 = latp.tile([B, LIVE], F32)
                for j, (n0, n) in enumerate(NSPL):
                    nc.vector.tensor_copy(out=lat1[:, n0:n0 + n], in_=pe[j])
                nc.sync.dma_start(out=ar_in[:, :], in_=lat1)

            post_ctx = tc.tile_pool(name="post", bufs=1)
            post = post_ctx.__enter__()
            # deferred big loads on sync queue (start after last W chunk)
            lmb_sb = post.tile([128, 4 * 3 * NLM], F32)
            nc.sync.dma_start(out=lmb_sb, in_=lmb_p[:, :])
            tsl_sb = post.tile([1, 3 * VS], F32)
            nc.scalar.dma_start(out=tsl_sb, in_=tsl_p[:, :])
            bsl_sb = post.tile([128, 4, 3 * VS], F32)
            nc.sync.dma_start(
                out=bsl_sb, in_=bsl_p.ap().rearrange("p (c n) -> p c n", n=3 * VS))

            # PE-warm dummies: keep the HAM clock up during the collective
            dum = minip.tile([B, LIVE], F32, name="dum", tag="dum")
            for _ in range(40):
                nc.tensor.matmul(dum, lhsT=ones8, rhs=cst[:, 0:LIVE],
                                 start=True, stop=True, skip_group_check=True)
            nc.gpsimd.collective_compute(
                "AllGather", ALU.bypass, replica_groups=grp,
                ins=[ar_in.ap().opt()], outs=[ar_out.ap().opt()])
            slabs = post.tile([B, NCORES, LIVE], F32)
            nc.sync.dma_start(
                out=slabs, in_=ar_out.ap().rearrange("c b l -> b c l"))
            s4 = post.tile([B, 4, LIVE], F32)
            for c in range(4):
                nc.vector.tensor_tensor(
                    out=s4[:, c, :], in0=slabs[:, 2 * c, :],
                    in1=slabs[:, 2 * c + 1, :], op=ALU.add)
            lat = latp.tile([B, LIVE], F32)
            nc.vector.tensor_tensor(
                out=lat, in0=s4[:, 0, :], in1=s4[:, 1, :], op=ALU.add)
            nc.vector.tensor_tensor(
                out=lat, in0=lat, in1=s4[:, 2, :], op=ALU.add)
            nc.vector.tensor_tensor(
                out=lat, in0=lat, in1=s4[:, 3, :], op=ALU.add)

            # ---------------- spT via PE transpose ----------------
            spT = []
            with tc.tile_pool(name="trp", bufs=1, space="PSUM") as trp:
                for (c0, kw) in KSPL:
                    pt = trp.tile([128, B], F32, name=f"pt{c0}", tag=f"pt{c0}")
                    nc.tensor.transpose(
                        out=pt[0:kw, :], in_=lat[:, c0:c0 + kw], identity=eye_sb
                    )
                    st = latp.tile([128, B], F32, name=f"spT{c0}")
                    nc.vector.tensor_copy(out=st[0:kw, :], in_=pt[0:kw, :])
                    spT.append(st)

            g = Geo(nc, geop)
            g2 = Geo(nc, geop, rows=128)

            # ---------------- mini-GEMMs: vmean, eye means, landmarks ----------
            pvm = minip.tile([B, 3], F32, name="pvm", tag="pvm")
            pem = minip.tile([B, 6], F32, name="pem", tag="pem")
            plm = minip.tile([B, 3 * NLM], F32, name="plm", tag="plm")
            for ki, (k0, kw) in enumerate(KSPL):
                nc.tensor.matmul(pvm, lhsT=spT[ki][:kw, :], rhs=bm_sb[:kw, ki * 3:ki * 3 + 3],
                                 start=(ki == 0), stop=False)
            nc.tensor.matmul(pvm, lhsT=ones1, rhs=cst[:, CTM:CTM + 3],
                             start=False, stop=True)
            for ki, (k0, kw) in enumerate(KSPL):
                nc.tensor.matmul(pem, lhsT=spT[ki][:kw, :], rhs=em_sb[:kw, ki * 6:ki * 6 + 6],
                                 start=(ki == 0), stop=False)
            nc.tensor.matmul(pem, lhsT=ones1, rhs=cst[:, CTE:CTE + 6],
                             start=False, stop=True)
            for ki, (k0, kw) in enumerate(KSPL):
                nc.tensor.matmul(plm, lhsT=spT[ki][:kw, :],
                                 rhs=lmb_sb[:kw, ki * 3 * NLM:(ki + 1) * 3 * NLM],
                                 start=(ki == 0), stop=False)
            nc.tensor.matmul(plm, lhsT=ones1, rhs=cst[:, CTL:CTL + 3 * NLM],
                             start=False, stop=True)
            vms = geop.tile([B, 3], F32)
            nc.vector.tensor_copy(out=vms, in_=pvm)

            # ---------------- face rotation (critical path only) ---------------
            aa_face = lat[:, C_ROT:C_ROT + 3]
            [Rf] = axis_angle_R_multi(nc, [(g, aa_face, "f_", halfpi[:B, :])])
            fs = g.t()
            nc.vector.tensor_scalar_add(out=fs, in0=lat[:, C_S:C_S + 1], scalar1=1.0)
            Rs = geop.tile([B, 9], F32)
            nc.vector.tensor_scalar_mul(out=Rs, in0=Rf, scalar1=fs)
            off = geop.tile([B, 3], F32)
            for i in range(3):
                t = g.mul(vms[:, 0:1], Rs[:, i:i + 1])
                t = g.mac(vms[:, 1:2], Rs[:, 3 + i:4 + i], t)
                t = g.mac(vms[:, 2:3], Rs[:, 6 + i:7 + i], t)
                nc.vector.tensor_tensor(
                    out=off[:, i:i + 1], in0=lat[:, C_T + i:C_T + i + 1], in1=t,
                    op=ALU.subtract,
                )

            # ---------------- blendshape slice + rotate + project -------------
            pg = post.tile([B, 6, VS], F32)
            rt_c = pg[:, 0:3, :]
            vs_t = post.tile([B, 3, VS], F32)
            with tc.tile_pool(name="bpsum", bufs=3, space="PSUM") as bpsum:
                for p in range(3):
                    pv = bpsum.tile([B, VS], F32)
                    for ki, (k0, kw) in enumerate(KSPL):
                        nc.tensor.matmul(
                            pv, lhsT=spT[ki][:kw, :],
                            rhs=bsl_sb[:kw, ki, p * VS:(p + 1) * VS],
                            start=(ki == 0), stop=False,
                        )
                    nc.tensor.matmul(pv, lhsT=ones1, rhs=tsl_sb[:, p * VS:(p + 1) * VS],
                                     start=False, stop=True)
                    nc.vector.tensor_copy(out=vs_t[:, p, :], in_=pv)
                _rotate3(nc, rt_c, vs_t, Rs, off, VS)

            # projection of own slice into pg planes 3..5
            img_c = pg[:, 3:6, :]
            for i in (2, 0, 1):
                nc.vector.tensor_scalar(
                    out=img_c[:, i, :], in0=rt_c[:, 0, :],
                    scalar1=cam[:, 4 * i:4 * i + 1], scalar2=cam[:, 4 * i + 3:4 * i + 4],
                    op0=ALU.mult, op1=ALU.add,
                )
                for l in (1, 2):
                    nc.vector.scalar_tensor_tensor(
                        out=img_c[:, i, :], in0=rt_c[:, l, :],
                        scalar=cam[:, 4 * i + l:4 * i + l + 1],
                        in1=img_c[:, i, :], op0=ALU.mult, op1=ALU.add,
                    )
            az_ = post.tile([B, VS], F32)
            nc.scalar.activation(out=az_, in_=img_c[:, 2, :], func=ACTF.Abs)
            nc.vector.tensor_scalar_max(out=az_, in0=az_, scalar1=1e-3)
            sg = post.tile([B, VS], F32)
            nc.vector.tensor_scalar(
                out=sg, in0=img_c[:, 2, :], scalar1=0.0, scalar2=None, op0=ALU.is_ge
            )
            nc.vector.tensor_scalar(
                out=sg, in0=sg, scalar1=2.0, scalar2=1.0,
                op0=ALU.mult, op1=ALU.subtract,
            )
            nc.vector.tensor_tensor(out=sg, in0=sg, in1=az_, op=ALU.mult)
            nc.vector.reciprocal(out=az_, in_=sg)
            nc.vector.tensor_tensor(
                out=img_c[:, 0, :], in0=img_c[:, 0, :], in1=az_, op=ALU.mult
            )
            nc.vector.tensor_tensor(
                out=img_c[:, 1, :], in0=img_c[:, 1, :], in1=az_, op=ALU.mult
            )
            nc.sync.dma_start(out=ag_in[:, :, :], in_=pg)
            nc.gpsimd.collective_compute(
                "AllGather", ALU.bypass, replica_groups=grp,
                ins=[ag_in.ap().opt()], outs=[ag_out.ap().opt()])

            # ---------------- geometry (hides under AGs) ----------------------
            aa2 = geop.tile([128, 3], F32)
            nc.vector.memset(aa2, 0.0)
            nc.vector.tensor_copy(out=aa2[0:B, 0:2], in_=lat[:, C_LROT:C_LROT + 2])
            nc.gpsimd.dma_start(out=aa2[B:128, 0:2], in_=lat[:, C_RROT:C_RROT + 2])
            [R2] = axis_angle_R_multi(nc, [(g2, aa2, "e_", halfpi)])
            em_raw = geop.tile([B, 6], F32)
            nc.vector.tensor_copy(out=em_raw, in_=pem)
            lm_raw = geop.tile([B, 3, NLM], F32)
            nc.vector.tensor_copy(out=lm_raw, in_=plm)
            lm_t = geop.tile([B, 3, NLM], F32)
            _rotate3(nc, lm_t, lm_raw, Rs, off, NLM)
            for i in range(3):
                nc.scalar.dma_start(
                    out=out_p[:, i, 2 * VM:2 * VM + 68], in_=lm_t[:, i, 0:68]
                )
            fc = geop.tile([B, 3], F32)
            for i in range(3):
                t4 = g.add(lm_t[:, i, 68:69], lm_t[:, i, 69:70])
                t4 = g.add(t4, lm_t[:, i, 70:71])
                t4 = g.add(t4, lm_t[:, i, 71:72])
                t2 = g.add(lm_t[:, i, 72:73], lm_t[:, i, 73:74])
                o = g.t()
                nc.vector.tensor_scalar_mul(out=o, in0=t4, scalar1=0.125)
                nc.vector.scalar_tensor_tensor(
                    out=fc[:, i:i + 1], in0=t2, scalar=0.25, in1=o,
                    op0=ALU.mult, op1=ALU.add,
                )

            # eye centres: affine of raw means (stacked l/r on 128 rows)
            raw3 = geop.tile([128, 3], F32)
            nc.vector.tensor_copy(out=raw3[0:B, :], in_=em_raw[:, 0:3])
            nc.gpsimd.dma_start(out=raw3[B:128, :], in_=em_raw[:, 3:6])
            Rs128 = geop.tile([128, 9], F32)
            nc.vector.tensor_copy(out=Rs128[0:B, :], in_=Rs)
            nc.gpsimd.dma_start(out=Rs128[B:128, :], in_=Rs)
            off128 = geop.tile([128, 3], F32)
            nc.vector.tensor_copy(out=off128[0:B, :], in_=off)
            nc.gpsimd.dma_start(out=off128[B:128, :], in_=off)
            c3 = geop.tile([128, 3], F32)
            for i in range(3):
                o = g2.t()
                nc.vector.scalar_tensor_tensor(
                    out=o, in0=raw3[:, 0:1], scalar=Rs128[:, i:i + 1],
                    in1=off128[:, i:i + 1], op0=ALU.mult, op1=ALU.add,
                )
                o = g2.mac(raw3[:, 1:2], Rs128[:, 3 + i:4 + i], o)
                o = g2.mac(raw3[:, 2:3], Rs128[:, 6 + i:7 + i], o)
                nc.vector.tensor_copy(out=c3[:, i:i + 1], in_=o)

            gz = geop.tile([128, 3], F32)
            nc.vector.tensor_scalar_mul(out=gz, in0=R2[:, 6:9], scalar1=GAZE_DIR)
            rc64 = geop.tile([B, 3], F32)
            nc.gpsimd.dma_start(out=rc64, in_=c3[B:128, :])
            rg64 = geop.tile([B, 3], F32)
            nc.gpsimd.dma_start(out=rg64, in_=gz[B:128, :])
            lc = c3[0:B, :]
            lg = gz[0:B, :]
            rc = rc64
            rg = rg64

            # gaze intersection (Cramer)
            d = [g.sub(rc[:, i:i + 1], lc[:, i:i + 1]) for i in range(3)]
            c0 = [lg[:, i:i + 1] for i in range(3)]
            c1 = []
            for i in range(3):
                o = g.t()
                nc.vector.tensor_scalar_mul(out=o, in0=rg[:, i:i + 1], scalar1=-1.0)
                c1.append(o)
            c2 = list(g.cross3(rg[:, 0:1], rg[:, 1:2], rg[:, 2:3],
                               lg[:, 0:1], lg[:, 1:2], lg[:, 2:3]))
            w = g.cross3(*c1, *c2)
            det = g.dot3(*c0, *w)
            num0 = g.dot3(*d, *w)
            w2 = g.cross3(*d, *c2)
            num1 = g.dot3(*c0, *w2)
            rdet = g.t()
            nc.vector.reciprocal(out=rdet, in_=det)
            sol0 = g.mul(num0, rdet)
            sol1 = g.mul(num1, rdet)
            gpl = geop.tile([B, 3], F32)
            gpr = geop.tile([B, 3], F32)
            gpm = geop.tile([B, 3], F32)
            for i in range(3):
                nc.vector.scalar_tensor_tensor(
                    out=gpl[:, i:i + 1], in0=lg[:, i:i + 1], scalar=sol0,
                    in1=lc[:, i:i + 1], op0=ALU.mult, op1=ALU.add,
                )
                nc.vector.scalar_tensor_tensor(
                    out=gpr[:, i:i + 1], in0=rg[:, i:i + 1], scalar=sol1,
                    in1=rc[:, i:i + 1], op0=ALU.mult, op1=ALU.add,
                )
            nc.vector.tensor_tensor(out=gpm, in0=gpl, in1=gpr, op=ALU.add)
            nc.vector.tensor_scalar_mul(out=gpm, in0=gpm, scalar1=0.5)
            dff = geop.tile([B, 3], F32)
            nc.vector.tensor_tensor(out=dff, in0=gpl, in1=gpr, op=ALU.subtract)
            nc.vector.tensor_tensor(out=dff, in0=dff, in1=dff, op=ALU.mult)
            d2 = g.t()
            nc.vector.tensor_reduce(out=d2, in_=dff, axis=AX.X, op=ALU.add)
            dist = g.t()
            nc.scalar.activation(out=dist, in_=d2, func=ACTF.Sqrt)
            farl = geop.tile([B, 3], F32)
            farr = geop.tile([B, 3], F32)
            for i in range(3):
                nc.vector.scalar_tensor_tensor(
                    out=farl[:, i:i + 1], in0=lg[:, i:i + 1], scalar=1000.0,
                    in1=lc[:, i:i + 1], op0=ALU.mult, op1=ALU.add,
                )
                nc.vector.scalar_tensor_tensor(
                    out=farr[:, i:i + 1], in0=rg[:, i:i + 1], scalar=1000.0,
                    in1=rc[:, i:i + 1], op0=ALU.mult, op1=ALU.add,
                )

            # tail assembly [B, 3, 11]
            tail = geop.tile([B, 3, 11], F32)

            def _cp(k, out, in_):
                e = k % 3
                if e == 0:
                    nc.vector.tensor_copy(out=out, in_=in_)
                elif e == 1:
                    nc.scalar.copy(out=out, in_=in_)
                else:
                    nc.gpsimd.tensor_copy(out=out, in_=in_)

            for i in range(3):
                pieces = [
                    lc[:, i:i + 1], rc[:, i:i + 1], fc[:, i:i + 1],
                    gpl[:, i:i + 1], gpr[:, i:i + 1], gpm[:, i:i + 1],
                    farl[:, i:i + 1], farr[:, i:i + 1],
                    lg[:, i:i + 1], rg[:, i:i + 1], dist,
                ]
                for j, src in enumerate(pieces):
                    _cp(i * 11 + j, tail[:, i, j:j + 1], src)
            for i in range(3):
                nc.scalar.dma_start(
                    out=out_p[:, i, 2 * VM + 68:NOUT], in_=tail[:, i, :]
                )

            # ---------------- scatter gathered slices dram->dram --------------
            for j, c in enumerate(range(NCORES)):
                c0_ = c * VS
                nb = min(VM, c0_ + VS) - c0_
                e1 = nc.sync if c % 2 == 0 else nc.scalar
                e2 = nc.scalar if c % 2 == 0 else nc.sync
                e1.dma_start(
                    out=out_p[:, :, c0_:c0_ + nb],
                    in_=ag_out.ap()[c][:, 0:3, 0:nb],
                )
                e2.dma_start(
                    out=out_p[:, :, VM + c0_:VM + c0_ + nb],
                    in_=ag_out.ap()[c][:, 3:6, 0:nb],
                )
            post_ctx.__exit__(None, None, None)
    _legalize_waits(nc)
    return nc


def _prep(inputs):
    x = np.ascontiguousarray(np.asarray(inputs["x"], np.float32).reshape(B, DIN))
    W = np.asarray(inputs["enc_W"], np.float32)
    b = np.asarray(inputs["enc_b"], np.float32)
    tmpl = np.asarray(inputs["v_template"], np.float32)        # [V, 3]
    basis = np.asarray(inputs["shape_basis"], np.float32)      # [400, V, 3]
    cam = np.ascontiguousarray(
        np.asarray(inputs["camera_parameters"], np.float32).reshape(B, 12))
    lm = np.asarray(inputs["landmarks"])
    mlm = np.asarray(inputs["masked_landmarks"])
    fmask = np.asarray(inputs["face_mask"])
    lmask = np.asarray(inputs["left_eyeball_mask"])
    rmask = np.asarray(inputs["right_eyeball_mask"])

    live = list(range(400)) + list(range(545, 556))
    Wl = np.concatenate([W[:, live], np.zeros((DIN, 1), np.float32)], axis=1)  # [DIN, 412]
    bl = np.concatenate([b[live], np.zeros(1, np.float32)])

    fl_idx = [int(fmask[i]) for i in mlm]
    idx4 = [int(lm[j]) for j in (19, 22, 25, 28)]
    idx2 = [int(lm[j]) for j in (14, 18)]
    lm_all = fl_idx + idx4 + idx2  # 74

    cst = np.zeros((1, CSTW), np.float32)
    cst[0, CB:CB + LIVE] = bl
    cst[0, CO8:CO8 + B] = 1.0 / NCORES
    cst[0, CO1:CO1 + B] = 1.0
    cst[0, CTM:CTM + 3] = tmpl.mean(axis=0)
    cst[0, CTE:CTE + 3] = tmpl[lmask].mean(axis=0)
    cst[0, CTE + 3:CTE + 6] = tmpl[rmask].mean(axis=0)
    cst[0, CTL:CTL + 3 * NLM] = tmpl[lm_all].T.reshape(-1)  # plane-major [3, 74]

    eye64 = np.eye(B, dtype=np.float32)

    bmean_full = basis.mean(axis=1)            # [400, 3]
    el = basis[:, lmask].mean(axis=1)
    er = basis[:, rmask].mean(axis=1)
    bm = np.zeros((128, 12), np.float32)
    em = np.zeros((128, 24), np.float32)
    lmb = np.zeros((128, 4 * 3 * NLM), np.float32)
    bas_lm = basis[:, lm_all].transpose(0, 2, 1).reshape(400, 3 * NLM)
    for ki, (k0, kw) in enumerate(KSPL):
        bm[:kw, ki * 3:ki * 3 + 3] = bmean_full[k0:k0 + kw]
        em[:kw, ki * 6:ki * 6 + 3] = el[k0:k0 + kw]
        em[:kw, ki * 6 + 3:ki * 6 + 6] = er[k0:k0 + kw]
        lmb[:kw, ki * 3 * NLM:(ki + 1) * 3 * NLM] = bas_lm[k0:k0 + kw]

    in_maps = []
    basis_pm = basis.transpose(0, 2, 1)  # [400, 3, V]
    tmpl_pm = tmpl.T                     # [3, V]
    for c in range(NCORES):
        k0 = c * KSH
        xs = x[:, k0:k0 + KSH].reshape(B, KTILES, 128).transpose(2, 1, 0)
        xpk = np.ascontiguousarray(xs.reshape(128, KTILES * B))
        ws = Wl[k0:k0 + KSH].reshape(KTILES, 128, LIVE).transpose(1, 0, 2)
        wpk = np.ascontiguousarray(ws.reshape(128, KTILES * LIVE))
        v0 = c * VS
        v1 = min(V, v0 + VS)
        nb = v1 - v0
        bsl = np.zeros((128, 4, 3, VS), np.float32)
        tsl = np.zeros((1, 3, VS), np.float32)
        for ki, (kk0, kw) in enumerate(KSPL):
            bsl[:kw, ki, :, :nb] = basis_pm[kk0:kk0 + kw, :, v0:v1]
        tsl[0, :, :nb] = tmpl_pm[:, v0:v1]
        in_maps.append({
            "xp": xpk,
            "wp": wpk,
            "cst": cst,
            "eye64": eye64,
            "bm": bm,
            "em": em,
            "lmb": lmb,
            "bsl": np.ascontiguousarray(bsl.reshape(128, 4 * 3 * VS)),
            "tsl": np.ascontiguousarray(tsl.reshape(1, 3 * VS)),
            "cam": cam,
        })
    return in_maps


def _run(inputs, trace=False):
    in_maps = _prep(inputs)
    nc = build_graph()
    res = run_bass_kernel_spmd(
        nc, in_maps, core_ids=list(range(NCORES)), trace=trace
    )
    out = res.results[0]["out"]  # [B, 3, NOUT]
    return np.ascontiguousarray(out.transpose(0, 2, 1)), res


def kernel(**inputs):
    out, _ = _run(inputs, trace=False)
    return out
